# revision 2
# baseline (speedup 1.0000x reference)
# Trainium2 Bass kernel for nn_AttentionBlock (local 7x11 windowed attention).
# V2: flattened cross-batch pipeline, PSUM bank parity, fp8-DoubleRow q/k path,
# tight exp APs, split DMA queues, norm/proj chain spread across tile slots.
import numpy as np
import ml_dtypes
from collections import deque

B, H, WG, C, HEADS = 32, 8, 64, 256, 8
HK, WK = 7, 11
N = H * WG              # 512
HD = C // HEADS         # 32
SCALE = float(HD) ** -0.5
NCORES = 8
BPC = B // NCORES       # 4
WT = 16                 # key-chunk width (grid cols)
NKC = WG // WT          # 4
HALO = WK // 2          # 5

# n' = w*8 + h  ->  n = h*64 + w
PERM = np.array([(i % H) * WG + (i // H) for i in range(N)], dtype=np.int64)


def _kc_qwin(kc):
    c0 = max(0, WT * kc - HALO)
    c1 = min(WG, WT * kc + WT + HALO)
    return c0 * H, c1 * H


QW = [_kc_qwin(kc) for kc in range(NKC)]          # real [qw0, qw1)
QA = [qw0 - (qw0 % 64) for qw0, _ in QW]          # aligned start
QE = [qw1 + (-qw1) % 64 for _, qw1 in QW]         # aligned end
WP = [QE[kc] - QA[kc] for kc in range(NKC)]       # padded width


def _segs(kc):
    return [(a, a + 64) for a in range(QA[kc], QE[kc], 64)]


SEGS = [_segs(kc) for kc in range(NKC)]
LASTKC = {}
for kc in range(NKC):
    for (sa, sb_) in SEGS[kc]:
        LASTKC[sa // 128] = kc

# packed-constant layout (columns of constsT [128, CW] bf16)
M01OFF = []
_off = 0
for _kc in range(NKC):
    M01OFF.append(_off)
    _off += 2 * WP[_kc]
M01END = _off            # split point for the early consts DMA
SELOFF = _off            # sel [16, 1024] at rows 0:16
ONESCOFF = SELOFF + 1024  # ones column [128, 1]
ONESROFF = ONESCOFF + 1   # ones row [1, 512] (row 0)
ZROWOFF = ONESROFF + 512  # zero row [1, 128] (row 0)
CW = ZROWOFF + 128

_NC_CACHE = {}

# global tile indices (ti within batch) whose mask-multiply runs on GPSIMD
_POOL_MULS = {0, 2, 4, 6, 8, 10, 12, 14}

LAG = 4

_WAIT_CAPS = {
    k: 1
    for k in (
        "InstMatmult", "InstLdweights", "InstActivation", "InstTensorTensor",
        "InstTensorCopy", "InstDMACopy", "InstDrain", "InstCustomDveAnt",
        "InstTensorScalarPtr", "InstMemset", "InstTensorReduce",
        "InstReciprocal",
    )
}
_NOP_WAIT_CAP = 1


def _split_waits(nc):
    import concourse.mybir as mybir

    ctr = [0]
    for fn in nc.m.functions:
        for bb in fn.blocks:
            out = []
            for ins in bb.instructions:
                cap = _WAIT_CAPS.get(ins.__class__.__name__)
                si = getattr(ins, "sync_info", None)
                waits = list(si.on_wait) if si is not None else []
                if cap is not None and len(waits) > cap:
                    excess = waits[:-cap] if cap else waits
                    keep = waits[-cap:] if cap else []
                    while excess:
                        chunk = excess[:_NOP_WAIT_CAP]
                        excess = excess[_NOP_WAIT_CAP:]
                        w = mybir.InstEventSemaphore(
                            name=f"wsplit{ctr[0]}", ins=[], outs=[]
                        )
                        ctr[0] += 1
                        w.engine = ins.engine
                        w.sync_info = mybir.SyncInfo(
                            on_wait=chunk, on_update=[]
                        )
                        out.append(w)
                    ins.sync_info = mybir.SyncInfo(
                        on_wait=keep, on_update=list(si.on_update)
                    )
                out.append(ins)
            bb.instructions = out


def _build_nc(split_waits=True):
    key = ("nc", split_waits)
    if key in _NC_CACHE:
        return _NC_CACHE[key]
    import concourse.bass as bass
    import concourse.mybir as mybir
    import concourse.tile as tile
    import contextlib

    f32 = mybir.dt.float32
    bf16 = mybir.dt.bfloat16
    f8 = mybir.dt.float8e4
    EXP = mybir.ActivationFunctionType.Exp
    DR = mybir.MatmulPerfMode.DoubleRow

    nc = bass.Bass("TRN2")

    xTd = nc.dram_tensor("xT", [128, BPC * 1024], bf16, kind="ExternalInput")
    x8d = nc.dram_tensor("x8", [128, BPC * 1024], f8, kind="ExternalInput")
    w8d = nc.dram_tensor("w8", [128, 1024], f8, kind="ExternalInput")
    wTd = nc.dram_tensor("wT", [128, 1024], bf16, kind="ExternalInput")
    constsT = nc.dram_tensor("constsT", [128, CW], bf16, kind="ExternalInput")
    identT = nc.dram_tensor("identT", [128, 128], f32, kind="ExternalInput")
    out = nc.dram_tensor("out", [BPC, N, C], f32, kind="ExternalOutput")

    with tile.TileContext(nc) as tc:
        with contextlib.ExitStack() as ctx:
            singles = ctx.enter_context(tc.tile_pool(name="singles", bufs=1))
            sb = ctx.enter_context(tc.tile_pool(name="sb", bufs=2))
            psp = ctx.enter_context(tc.tile_pool(name="ps", bufs=1, space="PSUM"))

            # ---- PSUM banks: 3 rotating + 4 avT (batch parity) + 1 shared ----
            rot = [psp.tile([128, 512], f32, name=f"rot{i}") for i in range(3)]
            avb = [psp.tile([128, 512], f32, name=f"avp{i}") for i in range(4)]
            sums_t = psp.tile([128, 32], f32, name="sums")
            rotc = [0]

            def nxt_rot():
                t = rot[rotc[0] % 3]
                rotc[0] += 1
                return t

            # ---- singles + DMAs (two queues: sync critical, scalar bg) ----
            s_w8 = singles.tile([128, 1024], f8, name="s_w8")
            s_wT = singles.tile([128, 1024], bf16, name="s_wT")
            s_x8 = singles.tile([128, BPC * 1024], f8, name="s_x8")
            s_xT = singles.tile([128, BPC * 1024], bf16, name="s_xT")
            s_consts = singles.tile([128, CW], bf16, name="s_consts")
            s_ident = singles.tile([128, 128], f32, name="s_ident")

            nc.sync.dma_start(out=s_w8, in_=w8d[:, :])
            nc.sync.dma_start(out=s_x8[:, 0:1024], in_=x8d[:, 0:1024])
            nc.sync.dma_start(out=s_wT, in_=wTd[:, :])
            nc.sync.dma_start(out=s_xT[:, 0:1024], in_=xTd[:, 0:1024])
            nc.sync.dma_start(
                out=s_consts[:, 0:M01END], in_=constsT[:, 0:M01END]
            )
            nc.sync.dma_start(out=s_x8[:, 1024:], in_=x8d[:, 1024:])
            nc.sync.dma_start(out=s_xT[:, 1024:], in_=xTd[:, 1024:])
            nc.scalar.dma_start(
                out=s_consts[:, M01END:CW], in_=constsT[:, M01END:CW]
            )
            nc.scalar.dma_start(out=s_ident, in_=identT[:, :])

            s_m01 = {
                kc: s_consts[:, M01OFF[kc]: M01OFF[kc] + 2 * WP[kc]]
                for kc in range(NKC)
            }
            s_sel = s_consts[0:16, SELOFF:SELOFF + 1024]
            s_onesc = s_consts[:, ONESCOFF:ONESCOFF + 1]
            s_onesr = s_consts[0:1, ONESROFF:ONESROFF + 512]
            s_zrow = s_consts[0:1, ZROWOFF:ZROWOFF + 128]
            s_wv = [s_wT[:, cc * 512: cc * 512 + 256] for cc in range(2)]
            s_wp = [s_wT[:, cc * 512 + 256: cc * 512 + 512] for cc in range(2)]

            st = {}  # per-batch state: qk8, v, avT_sb

            def qkv_piece(b, piece):
                x8r = s_x8.rearrange("p (b i n) -> p b i n", b=BPC, i=2)
                w8r = s_w8.rearrange("p (i f) -> p i f", i=2)
                if piece == "q8":
                    st[b] = {"v": None, "avT_sb": None}
                    st[b]["qk8"] = sb.tile(
                        [128, 3072], f8, tag="qk8", bufs=2, name=f"qk8_{b}"
                    )
                    if b < 2:
                        # zero plane read by the DoubleRow scores matmuls
                        nc.gpsimd.memset(st[b]["qk8"][:, 2048:3072], 0.0)
                if piece in ("q8", "k8"):
                    base = 0 if piece == "q8" else 2
                    for P in (base, base + 1):
                        pt = nxt_rot()
                        nc.tensor.matmul(
                            pt[:, 0:512],
                            lhsT=w8r[:, :, P * 128:(P + 1) * 128],
                            rhs=x8r[:, b],
                            start=True, stop=True, perf_mode=DR,
                        )
                        nc.vector.tensor_copy(
                            st[b]["qk8"][:, P * 512:(P + 1) * 512],
                            pt[:, 0:512],
                        )
                    # w8r free index f = P*128 + m maps to w_qkv row f
                else:
                    if piece == "v01":
                        st[b]["v"] = sb.tile(
                            [128, 1024], bf16, tag="v", bufs=2, name=f"v{b}"
                        )
                        k0 = 0
                    else:
                        k0 = 2
                    pt = nxt_rot()
                    for kcb in (k0, k0 + 1):
                        for cc in range(2):
                            nc.tensor.matmul(
                                pt[:, (kcb % 2) * 256:(kcb % 2) * 256 + 256],
                                lhsT=s_xT[:, b * 1024 + cc * 512 + kcb * 128:
                                          b * 1024 + cc * 512 + kcb * 128 + 128],
                                rhs=s_wv[cc][:, :],
                                start=(cc == 0), stop=(cc == 1),
                            )
                    nc.vector.tensor_copy(
                        st[b]["v"][:, k0 * 256:k0 * 256 + 512], pt[:, 0:512]
                    )

            def sums_preclear():
                nc.tensor.matmul(
                    sums_t[:, :], lhsT=s_zrow[:, :], rhs=s_onesr[:, 0:32],
                    start=True, stop=True, skip_group_check=True,
                )

            def preclears(b):
                par = (b % 2) * 2
                for i in range(2):
                    nc.tensor.matmul(
                        avb[par + i][:, :], lhsT=s_zrow[:, :], rhs=s_onesr[:, :],
                        start=True, stop=True, skip_group_check=True,
                    )
                st[b]["avT_sb"] = [
                    sb.tile([128, 512], bf16, tag="av", bufs=4, name=f"av{b}_{i}")
                    for i in range(2)
                ]

            tiles = [(kc, g) for kc in range(NKC) for g in range(4)]

            def tile_front(b, ti):
                kc, g = tiles[ti]
                qa0 = QA[kc]
                qw0, qw1 = QW[kc]
                pad = qw0 - qa0
                Wq = qw1 - qw0
                Wp = WP[kc]
                p_s = nxt_rot()
                r3 = st[b]["qk8"].rearrange("p (pl c) -> p pl c", pl=3)
                for i in range(2):
                    # lhsT planes: (k block fc=2+i, zeros); rhs planes:
                    # (q block fc=i, next block x 0) -- zero-padded DoubleRow
                    nc.tensor.matmul(
                        p_s[:, i * 256: i * 256 + Wp],
                        lhsT=r3[32 * g:32 * g + 32, 1:3,
                                i * 512 + kc * 128:i * 512 + kc * 128 + 128],
                        rhs=r3[32 * g:32 * g + 32, 0:2,
                               i * 512 + qa0:i * 512 + qa0 + Wp],
                        start=True, stop=True,
                        tile_position=(32 * g, 0), perf_mode=DR,
                    )
                e_t = sb.tile([128, 2 * Wp], bf16, tag="eT", bufs=8, name=f"eT{b}_{ti}")
                if b == 0:
                    # full width: seeds every eT pool slot with finite values
                    nc.scalar.activation(
                        e_t.rearrange("p (j s) -> p j s", j=2),
                        p_s.rearrange("p (j s) -> p j s", j=2)[:, :, :Wp],
                        EXP, scale=SCALE / 256.0,
                    )
                else:
                    # tight: pad cols keep stale-but-finite values; the mask
                    # multiply zeroes them
                    nc.scalar.activation(
                        e_t.rearrange("p (j s) -> p j s", j=2)[:, :, pad:pad + Wq],
                        p_s.rearrange("p (j s) -> p j s", j=2)[:, :, pad:pad + Wq],
                        EXP, scale=SCALE / 256.0,
                    )
                p_t = sb.tile([128, 2 * Wp], bf16, tag="pT", bufs=8, name=f"pT{b}_{ti}")
                meng = nc.gpsimd if ti in _POOL_MULS else nc.vector
                meng.tensor_mul(p_t, e_t, s_m01[kc][:, :2 * Wp])
                return (b, kc, g, p_t)

            def consume(b, kc, g, p_t):
                par = (b % 2) * 2
                soff = (b % 2) * 32
                qw0, qw1 = QW[kc]
                qa0 = QA[kc]
                Wq = qw1 - qw0
                Wp = WP[kc]
                pad = qw0 - qa0
                for i in range(2):
                    h = g + 4 * i
                    j = g
                    nc.tensor.matmul(
                        avb[par + i][32 * j:32 * j + 32, qw0:qw1],
                        lhsT=st[b]["v"][:, kc * 256 + h * 32:
                                        kc * 256 + (h + 1) * 32],
                        rhs=p_t[:, i * Wp + pad:(i * Wp) + pad + Wq],
                        start=False, stop=(kc == NKC - 1),
                        tile_position=(0, 32 * j),
                        skip_group_check=True,
                    )
                    for (sa, sbnd) in SEGS[kc]:
                        qc = sa // 128
                        qcol = qc * 8 + h
                        nc.tensor.matmul(
                            sums_t[sa % 128: sa % 128 + 64,
                                   qcol:qcol + 1],
                            lhsT=p_t[:, i * Wp + (sa - qa0):
                                     i * Wp + (sbnd - qa0)],
                            rhs=s_onesc[:, :],
                            start=False, stop=(kc == LASTKC[qc]),
                            tile_position=(0, sa % 128),
                            skip_group_check=True,
                        )

            def norm_steps(b, qs):
                par = (b % 2) * 2
                ns = {}

                def s_recip():
                    ns["r_q"] = sb.tile([128, 16], f32, tag="rq", bufs=4, name=f"rq{b}_{qs}")
                    nc.vector.reciprocal(
                        ns["r_q"], sums_t[:, qs * 16:qs * 16 + 16]
                    )

                def s_tp():
                    ns["p_rT"] = nxt_rot()
                    nc.tensor.matmul(
                        ns["p_rT"][0:16, 0:128], lhsT=ns["r_q"], rhs=s_ident,
                        is_transpose=True, skip_group_check=True,
                    )

                def s_rtcopy():
                    ns["rT_sb"] = sb.tile([16, 128], bf16, tag="rT", bufs=4, name=f"rT{b}_{qs}")
                    nc.vector.tensor_copy(ns["rT_sb"], ns["p_rT"][0:16, 0:128])

                def s_selmm():
                    ns["p_rb"] = nxt_rot()
                    for half in range(2):
                        for qcl in range(2):
                            idx = (qs * 2 + half) * 2 + qcl
                            nc.tensor.matmul(
                                ns["p_rb"][:, half * 256 + qcl * 128:
                                           half * 256 + qcl * 128 + 128],
                                lhsT=s_sel[:, idx * 128:idx * 128 + 128],
                                rhs=ns["rT_sb"],
                                start=True, stop=True,
                            )

                def s_rbcopy():
                    ns["rb_sb"] = sb.tile(
                        [128, 512], bf16, tag="rb", bufs=4, name=f"rb{b}_{qs}"
                    )
                    nc.scalar.copy(ns["rb_sb"], ns["p_rb"][:, :])

                def s_mul(half):
                    nc.vector.tensor_mul(
                        st[b]["avT_sb"][half][:, qs * 256:qs * 256 + 256],
                        ns["rb_sb"][:, half * 256:half * 256 + 256],
                        avb[par + half][:, qs * 256:qs * 256 + 256],
                    )

                def s_proj(qcl):
                    qc = 2 * qs + qcl
                    ns[f"p_o{qcl}"] = nxt_rot()
                    for half in range(2):
                        nc.tensor.matmul(
                            ns[f"p_o{qcl}"][:, 0:256],
                            lhsT=st[b]["avT_sb"][half][:, qc * 128:
                                                       (qc + 1) * 128],
                            rhs=s_wp[half][:, :],
                            start=(half == 0), stop=(half == 1),
                        )

                def s_out(qcl, on_act):
                    qc = 2 * qs + qcl
                    o_sb = sb.tile([128, 256], f32, tag="osb", bufs=4, name=f"osb{b}_{qs}_{qcl}")
                    if on_act:
                        nc.scalar.copy(o_sb, ns[f"p_o{qcl}"][:, 0:256])
                    else:
                        nc.vector.tensor_copy(o_sb, ns[f"p_o{qcl}"][:, 0:256])
                    nc.sync.dma_start(
                        out=out[b, qc * 128:(qc + 1) * 128, :], in_=o_sb
                    )

                return [
                    s_recip, s_tp, s_rtcopy, s_selmm, s_rbcopy,
                    lambda: s_mul(0), lambda: s_mul(1),
                    lambda: s_proj(0), lambda: s_out(0, True),
                    lambda: s_proj(1), lambda: s_out(1, False),
                ]

            # ---- prologue: batch 0 qkv + preclears ----
            qkv_piece(0, "q8")
            qkv_piece(0, "k8")
            qkv_piece(0, "v01")
            qkv_piece(0, "v23")
            preclears(0)
            sums_preclear()

            # ---- flat pipeline over all (batch, tile) slots ----
            pend = []
            normq = deque()
            T_total = BPC * 16
            for T in range(T_total + LAG):
                if T < T_total:
                    b, ti = divmod(T, 16)
                    pend.append(tile_front(b, ti))
                    nb = b + 1
                    if nb < BPC:
                        if ti == 5:
                            qkv_piece(nb, "q8")
                        elif ti == 7:
                            qkv_piece(nb, "k8")
                        elif ti == 9:
                            qkv_piece(nb, "v01")
                        elif ti == 11:
                            qkv_piece(nb, "v23")
                        elif ti == 13:
                            preclears(nb)
                # drain BEFORE consume: keeps stream order around the
                # sums preclear; 2/slot when backlogged
                k = 2 if len(normq) > 6 else 1
                for _ in range(min(k, len(normq))):
                    normq.popleft()()
                if T >= LAG:
                    consume(*pend[T - LAG])
                    cb, cti = divmod(T - LAG, 16)
                    if cti == 11:
                        normq.extend(norm_steps(cb, 0))
                    elif cti == 15:
                        # strict stream order: recip(qs1) reads the sums
                        # bank, then the preclear for batch cb+1 rewrites it,
                        # then (next slot) consume(cb+1, 0) accumulates
                        steps = norm_steps(cb, 1)
                        steps[0]()
                        if cb + 1 < BPC:
                            sums_preclear()
                        normq.extend(steps[1:])
            while normq:
                normq.popleft()()

    if split_waits:
        _split_waits(nc)
    _NC_CACHE[key] = nc
    return nc


def _host_inputs(x, w_qkv, w_proj, mask_np):
    """Build per-core input maps (host-side reshapes/permutes only)."""
    bf16 = ml_dtypes.bfloat16
    f8 = ml_dtypes.float8_e4m3
    xp = np.ascontiguousarray(x[:, PERM, :])                      # [B, N, C]
    xTp = np.ascontiguousarray(np.transpose(xp, (0, 2, 1)))       # [B, C, N]

    # bf16 x: per batch block [cc0 512 | cc1 512]
    xT_blk = xTp.reshape(B, 2, 128, N)                            # [B,cc,128,N]
    # fp8 x, DoubleRow pairs: block [i0 512 | i1 512], c = 2p+i
    x8_blk = xTp.reshape(B, 128, 2, N).transpose(0, 2, 1, 3)      # [B,i,128,N]

    # w8: [p, i*512 + f] = 16*w_qkv[f, 2p+i]  (DoubleRow channel pairs)
    W16 = (16.0 * np.asarray(w_qkv[:512])).astype(np.float32)     # [512, 256]
    w8 = np.ascontiguousarray(
        W16.T.reshape(128, 2, 512).reshape(128, 1024)
    ).astype(f8)

    wvT = np.ascontiguousarray(w_qkv[512:].T).reshape(2, 128, 256)
    wpT = np.ascontiguousarray(w_proj.T).reshape(2, 128, 256)
    wT = np.concatenate(
        [wvT[0], wpT[0], wvT[1], wpT[1]], axis=1
    ).astype(bf16)                                                # [128, 1024]

    consts = np.zeros((128, CW), dtype=np.float32)
    m01p = (mask_np[PERM][:, PERM] == 0.0)
    for kc in range(NKC):
        qa0, qe1 = QA[kc], QE[kc]
        t = m01p[qa0:qe1, 128 * kc:128 * kc + 128].T.astype(np.float32)
        consts[:, M01OFF[kc]:M01OFF[kc] + 2 * WP[kc]] = np.concatenate(
            [t] * 2, axis=1
        )
    for qs in range(2):
        for half in range(2):
            for qcl in range(2):
                idx2 = (qs * 2 + half) * 2 + qcl
                for mm in range(128):
                    consts[qcl * 8 + half * 4 + mm // 32,
                           SELOFF + idx2 * 128 + mm] = 1.0
    consts[:, ONESCOFF] = 1.0
    consts[0, ONESROFF:ONESROFF + 512] = 1.0

    base = {
        "w8": w8,
        "wT": wT,
        "constsT": consts.astype(bf16),
        "identT": np.eye(128, dtype=np.float32),
    }
    in_maps = []
    for core in range(NCORES):
        mp = dict(base)
        sl = slice(core * BPC, (core + 1) * BPC)
        xc = xT_blk[sl]                                           # [BPC,2,128,N]
        mp["xT"] = np.ascontiguousarray(
            xc.transpose(0, 2, 1, 3).reshape(BPC, 128, 1024)
            .transpose(1, 0, 2).reshape(128, BPC * 1024)
        ).astype(bf16)
        x8c = x8_blk[sl]                                          # [BPC,i,128,N]
        mp["x8"] = np.ascontiguousarray(
            x8c.transpose(2, 0, 1, 3).reshape(128, BPC * 1024)
        ).astype(f8)
        in_maps.append(mp)
    return in_maps


def run_sharded(x, w_qkv, w_proj, b_proj, mask, trace=False):
    """Compile+run on 8 cores; returns (out_full, BassKernelResults)."""
    from concourse.bass_utils import run_bass_kernel_spmd

    x = np.asarray(x, dtype=np.float32)
    w_qkv = np.asarray(w_qkv, dtype=np.float32)
    w_proj = np.asarray(w_proj, dtype=np.float32)
    b_proj = np.asarray(b_proj, dtype=np.float32)
    mask_np = np.asarray(mask, dtype=np.float32).reshape(N, N)

    nc = _build_nc()
    in_maps = _host_inputs(x, w_qkv, w_proj, mask_np)

    res = run_bass_kernel_spmd(nc, in_maps, core_ids=list(range(NCORES)), trace=trace)

    out_full = np.empty((B, N, C), dtype=np.float32)
    for core in range(NCORES):
        od = res.results[core]["out"]          # [BPC, N, C], permuted rows
        for bi in range(BPC):
            out_full[core * BPC + bi][PERM, :] = od[bi]
    out_full += b_proj[None, None, :]
    return out_full, res


def kernel(x, w_qkv, w_proj, b_proj, mask):
    out, _ = run_sharded(x, w_qkv, w_proj, b_proj, mask, trace=False)
    return out


# revision 3
# speedup vs baseline: 1.1398x; 1.1398x over previous
# Trainium2 Bass kernel for nn_AttentionBlock (local 7x11 windowed attention).
# V2: flattened cross-batch pipeline, PSUM bank parity, fp8-DoubleRow q/k path,
# tight exp APs, split DMA queues, norm/proj chain spread across tile slots.
import numpy as np
import ml_dtypes
from collections import deque

B, H, WG, C, HEADS = 32, 8, 64, 256, 8
HK, WK = 7, 11
N = H * WG              # 512
HD = C // HEADS         # 32
SCALE = float(HD) ** -0.5
NCORES = 8
BPC = B // NCORES       # 4
WT = 16                 # key-chunk width (grid cols)
NKC = WG // WT          # 4
HALO = WK // 2          # 5

# n' = w*8 + h  ->  n = h*64 + w
PERM = np.array([(i % H) * WG + (i // H) for i in range(N)], dtype=np.int64)


def _kc_qwin(kc):
    c0 = max(0, WT * kc - HALO)
    c1 = min(WG, WT * kc + WT + HALO)
    return c0 * H, c1 * H


QW = [_kc_qwin(kc) for kc in range(NKC)]          # real [qw0, qw1)
QA = [qw0 - (qw0 % 64) for qw0, _ in QW]          # aligned start
QE = [qw1 + (-qw1) % 64 for _, qw1 in QW]         # aligned end
WP = [QE[kc] - QA[kc] for kc in range(NKC)]       # padded width


def _segs(kc):
    return [(a, a + 64) for a in range(QA[kc], QE[kc], 64)]


SEGS = [_segs(kc) for kc in range(NKC)]
LASTKC = {}
for kc in range(NKC):
    for (sa, sb_) in SEGS[kc]:
        LASTKC[sa // 128] = kc

# packed-constant layout (columns of constsT [128, CW] bf16)
SELOFF = 0                # sel [16, 1024] at rows 0:16
ONESCOFF = SELOFF + 1024  # ones column [128, 1]
ONESROFF = ONESCOFF + 1   # ones row [1, 512] (row 0)
ZROWOFF = ONESROFF + 512  # zero row [1, 128] (row 0)
CW = ZROWOFF + 128

# fp8 mask-bias consts c8 [128, C8W]: U8 [128, 2, 128] then V8 per kc
# [128, 2, WP[kc]].  bias = U8.T (x) V8 accumulated into the scores PSUM via
# a zero-slot DoubleRow matmul; masked cells get -43264 raw (= -29.9 after
# the exp scale SCALE/256), unmasked cells get exactly 0.
V8OFF = []
_o8 = 256
for _kc in range(NKC):
    V8OFF.append(_o8)
    _o8 += 2 * WP[_kc]
C8W = _o8

_NC_CACHE = {}

# global tile indices (ti within batch) whose mask-multiply runs on GPSIMD
_POOL_MULS = {0, 2, 4, 6, 8, 10, 12, 14}

LAG = 5

_WAIT_CAPS = {
    k: 1
    for k in (
        "InstMatmult", "InstLdweights", "InstActivation", "InstTensorTensor",
        "InstTensorCopy", "InstDMACopy", "InstDrain", "InstCustomDveAnt",
        "InstTensorScalarPtr", "InstMemset", "InstTensorReduce",
        "InstReciprocal",
    )
}
_NOP_WAIT_CAP = 1


def _split_waits(nc):
    import concourse.mybir as mybir

    ctr = [0]
    for fn in nc.m.functions:
        for bb in fn.blocks:
            out = []
            for ins in bb.instructions:
                cap = _WAIT_CAPS.get(ins.__class__.__name__)
                si = getattr(ins, "sync_info", None)
                waits = list(si.on_wait) if si is not None else []
                if cap is not None and len(waits) > cap:
                    excess = waits[:-cap] if cap else waits
                    keep = waits[-cap:] if cap else []
                    while excess:
                        chunk = excess[:_NOP_WAIT_CAP]
                        excess = excess[_NOP_WAIT_CAP:]
                        w = mybir.InstEventSemaphore(
                            name=f"wsplit{ctr[0]}", ins=[], outs=[]
                        )
                        ctr[0] += 1
                        w.engine = ins.engine
                        w.sync_info = mybir.SyncInfo(
                            on_wait=chunk, on_update=[]
                        )
                        out.append(w)
                    ins.sync_info = mybir.SyncInfo(
                        on_wait=keep, on_update=list(si.on_update)
                    )
                out.append(ins)
            bb.instructions = out


def _build_nc(split_waits=True):
    key = ("nc", split_waits)
    if key in _NC_CACHE:
        return _NC_CACHE[key]
    import concourse.bass as bass
    import concourse.mybir as mybir
    import concourse.tile as tile
    import contextlib

    f32 = mybir.dt.float32
    bf16 = mybir.dt.bfloat16
    f8 = mybir.dt.float8e4
    EXP = mybir.ActivationFunctionType.Exp
    DR = mybir.MatmulPerfMode.DoubleRow

    nc = bass.Bass("TRN2")

    xTd = nc.dram_tensor("xT", [128, BPC * 1024], bf16, kind="ExternalInput")
    x8d = nc.dram_tensor("x8", [128, BPC * 1024], f8, kind="ExternalInput")
    w8d = nc.dram_tensor("w8", [128, 1024], f8, kind="ExternalInput")
    wTd = nc.dram_tensor("wT", [128, 1024], bf16, kind="ExternalInput")
    constsT = nc.dram_tensor("constsT", [128, CW], bf16, kind="ExternalInput")
    c8d = nc.dram_tensor("c8", [128, C8W], f8, kind="ExternalInput")
    identT = nc.dram_tensor("identT", [128, 128], f32, kind="ExternalInput")
    out = nc.dram_tensor("out", [BPC, N, C], f32, kind="ExternalOutput")

    with tile.TileContext(nc) as tc:
        with contextlib.ExitStack() as ctx:
            singles = ctx.enter_context(tc.tile_pool(name="singles", bufs=1))
            sb = ctx.enter_context(tc.tile_pool(name="sb", bufs=2))
            psp = ctx.enter_context(tc.tile_pool(name="ps", bufs=1, space="PSUM"))

            # ---- PSUM banks: 3 rotating + 4 avT (batch parity) + 1 shared ----
            rot = [psp.tile([128, 512], f32, name=f"rot{i}") for i in range(3)]
            avb = [psp.tile([128, 512], f32, name=f"avp{i}") for i in range(4)]
            sums_t = psp.tile([128, 32], f32, name="sums")
            rotc = [0]

            def nxt_rot():
                t = rot[rotc[0] % 3]
                rotc[0] += 1
                return t

            # ---- singles + DMAs (two queues: sync critical, scalar bg) ----
            s_w8 = singles.tile([128, 1024], f8, name="s_w8")
            s_wT = singles.tile([128, 1024], bf16, name="s_wT")
            s_x8 = singles.tile([128, BPC * 1024], f8, name="s_x8")
            s_xT = singles.tile([128, BPC * 1024], bf16, name="s_xT")
            s_consts = singles.tile([128, CW], bf16, name="s_consts")
            s_c8 = singles.tile([128, C8W], f8, name="s_c8")
            s_ident = singles.tile([128, 128], f32, name="s_ident")

            nc.sync.dma_start(out=s_w8, in_=w8d[:, :])
            nc.sync.dma_start(out=s_x8[:, 0:1024], in_=x8d[:, 0:1024])
            nc.sync.dma_start(
                out=s_consts[:, ONESCOFF:CW], in_=constsT[:, ONESCOFF:CW]
            )
            nc.sync.dma_start(out=s_c8, in_=c8d[:, :])
            nc.sync.dma_start(out=s_wT, in_=wTd[:, :])
            nc.sync.dma_start(out=s_xT[:, 0:1024], in_=xTd[:, 0:1024])
            nc.sync.dma_start(out=s_x8[:, 1024:], in_=x8d[:, 1024:])
            nc.sync.dma_start(out=s_xT[:, 1024:], in_=xTd[:, 1024:])
            nc.scalar.dma_start(
                out=s_consts[:, 0:ONESCOFF], in_=constsT[:, 0:ONESCOFF]
            )
            nc.scalar.dma_start(out=s_ident, in_=identT[:, :])

            s_u8 = s_c8[:, 0:256].rearrange("p (i k) -> p i k", i=2)
            s_v8 = {
                kc: s_c8[:, V8OFF[kc]: V8OFF[kc] + 2 * WP[kc]].rearrange(
                    "p (i q) -> p i q", i=2
                )
                for kc in range(NKC)
            }
            s_sel = s_consts[0:16, SELOFF:SELOFF + 1024]
            s_onesc = s_consts[:, ONESCOFF:ONESCOFF + 1]
            s_onesr = s_consts[0:1, ONESROFF:ONESROFF + 512]
            s_zrow = s_consts[0:1, ZROWOFF:ZROWOFF + 128]
            s_wv = [s_wT[:, cc * 512: cc * 512 + 256] for cc in range(2)]
            s_wp = [s_wT[:, cc * 512 + 256: cc * 512 + 512] for cc in range(2)]

            st = {}  # per-batch state: qk8, v, avT_sb

            def qkv_piece(b, piece):
                x8r = s_x8.rearrange("p (b i n) -> p b i n", b=BPC, i=2)
                w8r = s_w8.rearrange("p (i f) -> p i f", i=2)
                if piece == "q8":
                    st[b] = {"v": None, "avT_sb": None}
                    st[b]["qk8"] = sb.tile(
                        [128, 3072], f8, tag="qk8", bufs=2, name=f"qk8_{b}"
                    )
                    if b < 2:
                        # zero plane read by the DoubleRow scores matmuls
                        nc.gpsimd.memset(st[b]["qk8"][:, 2048:3072], 0.0)
                if piece in ("q8", "k8"):
                    base = 0 if piece == "q8" else 2
                    for P in (base, base + 1):
                        pt = nxt_rot()
                        nc.tensor.matmul(
                            pt[:, 0:512],
                            lhsT=w8r[:, :, P * 128:(P + 1) * 128],
                            rhs=x8r[:, b],
                            start=True, stop=True, perf_mode=DR,
                        )
                        nc.vector.tensor_copy(
                            st[b]["qk8"][:, P * 512:(P + 1) * 512],
                            pt[:, 0:512],
                        )
                    # w8r free index f = P*128 + m maps to w_qkv row f
                else:
                    if piece == "v01":
                        st[b]["v"] = sb.tile(
                            [128, 1024], bf16, tag="v", bufs=2, name=f"v{b}"
                        )
                        k0 = 0
                    else:
                        k0 = 2
                    pt = nxt_rot()
                    for kcb in (k0, k0 + 1):
                        for cc in range(2):
                            nc.tensor.matmul(
                                pt[:, (kcb % 2) * 256:(kcb % 2) * 256 + 256],
                                lhsT=s_xT[:, b * 1024 + cc * 512 + kcb * 128:
                                          b * 1024 + cc * 512 + kcb * 128 + 128],
                                rhs=s_wv[cc][:, :],
                                start=(cc == 0), stop=(cc == 1),
                            )
                    nc.vector.tensor_copy(
                        st[b]["v"][:, k0 * 256:k0 * 256 + 512], pt[:, 0:512]
                    )

            def sums_preclear():
                nc.tensor.matmul(
                    sums_t[:, :], lhsT=s_zrow[:, :], rhs=s_onesr[:, 0:32],
                    start=True, stop=True, skip_group_check=True,
                )

            def preclears(b):
                par = (b % 2) * 2
                for i in range(2):
                    nc.tensor.matmul(
                        avb[par + i][:, :], lhsT=s_zrow[:, :], rhs=s_onesr[:, :],
                        start=True, stop=True, skip_group_check=True,
                    )
                st[b]["avT_sb"] = [
                    sb.tile([128, 512], bf16, tag="av", bufs=6, name=f"av{b}_{i}")
                    for i in range(2)
                ]

            tiles = [(kc, g) for kc in range(NKC) for g in range(4)]

            def tile_front(b, ti):
                kc, g = tiles[ti]
                qa0 = QA[kc]
                qw0, qw1 = QW[kc]
                pad = qw0 - qa0
                Wq = qw1 - qw0
                Wp = WP[kc]
                p_s = nxt_rot()
                r3 = st[b]["qk8"].rearrange("p (pl c) -> p pl c", pl=3)
                for i in range(2):
                    # additive mask bias via exact fp8 DoubleRow factorization
                    nc.tensor.matmul(
                        p_s[:, i * 256: i * 256 + Wp],
                        lhsT=s_u8,
                        rhs=s_v8[kc][:, :, :],
                        start=True, stop=False,
                        tile_position=(0, 0), perf_mode=DR,
                        skip_group_check=True,
                    )
                    # lhsT planes: (k block fc=2+i, zeros); rhs planes:
                    # (q block fc=i, next block x 0) -- zero-padded DoubleRow
                    nc.tensor.matmul(
                        p_s[:, i * 256: i * 256 + Wp],
                        lhsT=r3[32 * g:32 * g + 32, 1:3,
                                i * 512 + kc * 128:i * 512 + kc * 128 + 128],
                        rhs=r3[32 * g:32 * g + 32, 0:2,
                               i * 512 + qa0:i * 512 + qa0 + Wp],
                        start=False, stop=True,
                        tile_position=(32 * g, 0), perf_mode=DR,
                        skip_group_check=True,
                    )
                e_t = sb.tile([128, 2 * Wp], bf16, tag="eT", bufs=8, name=f"eT{b}_{ti}")
                nc.scalar.activation(
                    e_t.rearrange("p (j s) -> p j s", j=2),
                    p_s.rearrange("p (j s) -> p j s", j=2)[:, :, :Wp],
                    EXP, scale=SCALE / 256.0,
                )
                return (b, kc, g, e_t)

            def consume(b, kc, g, p_t):
                par = (b % 2) * 2
                soff = (b % 2) * 32
                qw0, qw1 = QW[kc]
                qa0 = QA[kc]
                Wq = qw1 - qw0
                Wp = WP[kc]
                pad = qw0 - qa0
                for i in range(2):
                    h = g + 4 * i
                    j = g
                    nc.tensor.matmul(
                        avb[par + i][32 * j:32 * j + 32, qw0:qw1],
                        lhsT=st[b]["v"][:, kc * 256 + h * 32:
                                        kc * 256 + (h + 1) * 32],
                        rhs=p_t[:, i * Wp + pad:(i * Wp) + pad + Wq],
                        start=False, stop=(kc == NKC - 1),
                        tile_position=(0, 32 * j),
                        skip_group_check=True,
                    )
                    for (sa, sbnd) in SEGS[kc]:
                        qc = sa // 128
                        qcol = qc * 8 + h
                        nc.tensor.matmul(
                            sums_t[sa % 128: sa % 128 + 64,
                                   qcol:qcol + 1],
                            lhsT=p_t[:, i * Wp + (sa - qa0):
                                     i * Wp + (sbnd - qa0)],
                            rhs=s_onesc[:, :],
                            start=False, stop=(kc == LASTKC[qc]),
                            tile_position=(0, sa % 128),
                            skip_group_check=True,
                        )

            def norm_steps(b, qs):
                par = (b % 2) * 2
                ns = {}

                def s_recip():
                    ns["r_q"] = sb.tile([128, 16], f32, tag="rq", bufs=6, name=f"rq{b}_{qs}")
                    nc.vector.reciprocal(
                        ns["r_q"], sums_t[:, qs * 16:qs * 16 + 16]
                    )

                def s_tp():
                    ns["p_rT"] = nxt_rot()
                    nc.tensor.matmul(
                        ns["p_rT"][0:16, 0:128], lhsT=ns["r_q"], rhs=s_ident,
                        is_transpose=True, skip_group_check=True,
                    )

                def s_rtcopy():
                    ns["rT_sb"] = sb.tile([16, 128], bf16, tag="rT", bufs=6, name=f"rT{b}_{qs}")
                    nc.vector.tensor_copy(ns["rT_sb"], ns["p_rT"][0:16, 0:128])

                def s_selmm():
                    ns["p_rb"] = nxt_rot()
                    for half in range(2):
                        for qcl in range(2):
                            idx = (qs * 2 + half) * 2 + qcl
                            nc.tensor.matmul(
                                ns["p_rb"][:, half * 256 + qcl * 128:
                                           half * 256 + qcl * 128 + 128],
                                lhsT=s_sel[:, idx * 128:idx * 128 + 128],
                                rhs=ns["rT_sb"],
                                start=True, stop=True,
                            )

                def s_rbcopy():
                    ns["rb_sb"] = sb.tile(
                        [128, 512], bf16, tag="rb", bufs=6, name=f"rb{b}_{qs}"
                    )
                    nc.vector.tensor_copy(
                        ns["rb_sb"][:, 0:256], ns["p_rb"][:, 0:256]
                    )
                    nc.scalar.copy(ns["rb_sb"][:, 256:512], ns["p_rb"][:, 256:512])

                def s_mul(half):
                    nc.vector.tensor_mul(
                        st[b]["avT_sb"][half][:, qs * 256:qs * 256 + 256],
                        ns["rb_sb"][:, half * 256:half * 256 + 256],
                        avb[par + half][:, qs * 256:qs * 256 + 256],
                    )

                def s_proj(qcl):
                    qc = 2 * qs + qcl
                    ns[f"p_o{qcl}"] = nxt_rot()
                    for half in range(2):
                        nc.tensor.matmul(
                            ns[f"p_o{qcl}"][:, 0:256],
                            lhsT=st[b]["avT_sb"][half][:, qc * 128:
                                                       (qc + 1) * 128],
                            rhs=s_wp[half][:, :],
                            start=(half == 0), stop=(half == 1),
                        )

                def s_out(qcl, on_act):
                    qc = 2 * qs + qcl
                    o_sb = sb.tile([128, 256], f32, tag="osb", bufs=6, name=f"osb{b}_{qs}_{qcl}")
                    if on_act:
                        nc.scalar.copy(o_sb, ns[f"p_o{qcl}"][:, 0:256])
                    else:
                        nc.vector.tensor_copy(o_sb, ns[f"p_o{qcl}"][:, 0:256])
                    nc.sync.dma_start(
                        out=out[b, qc * 128:(qc + 1) * 128, :], in_=o_sb
                    )

                return [
                    s_recip, s_tp, s_rtcopy, s_selmm, s_rbcopy,
                    lambda: s_mul(0), lambda: s_mul(1),
                    lambda: s_proj(0), lambda: s_out(0, qs == 1),
                    lambda: s_proj(1), lambda: s_out(1, False),
                ]

            # ---- prologue: batch 0 qkv + preclears ----
            qkv_piece(0, "q8")
            qkv_piece(0, "k8")
            preclears(0)
            sums_preclear()

            # ---- flat pipeline over all (batch, tile) slots ----
            pend = []
            normq = deque()
            T_total = BPC * 16
            for T in range(T_total + LAG):
                if T < T_total:
                    b, ti = divmod(T, 16)
                    pend.append(tile_front(b, ti))
                    if b == 0:
                        if ti == 1:
                            qkv_piece(0, "v01")
                        elif ti == 3:
                            qkv_piece(0, "v23")
                    nb = b + 1
                    if nb < BPC:
                        if ti == 5:
                            qkv_piece(nb, "q8")
                        elif ti == 7:
                            qkv_piece(nb, "k8")
                        elif ti == 9:
                            qkv_piece(nb, "v01")
                        elif ti == 11:
                            qkv_piece(nb, "v23")
                        elif ti == 13:
                            preclears(nb)
                # drain BEFORE consume: keeps stream order around the
                # sums preclear; 2/slot when backlogged
                k = 2 if len(normq) > 6 else 1
                for _ in range(min(k, len(normq))):
                    normq.popleft()()
                if T >= LAG:
                    consume(*pend[T - LAG])
                    cb, cti = divmod(T - LAG, 16)
                    if cti == 11:
                        normq.extend(norm_steps(cb, 0))
                    elif cti == 15:
                        # strict stream order: recip(qs1) reads the sums
                        # bank, then the preclear for batch cb+1 rewrites it,
                        # then (next slot) consume(cb+1, 0) accumulates
                        steps = norm_steps(cb, 1)
                        steps[0]()
                        if cb + 1 < BPC:
                            sums_preclear()
                        normq.extend(steps[1:])
            while normq:
                normq.popleft()()

    if split_waits:
        _split_waits(nc)
    _NC_CACHE[key] = nc
    return nc


def _host_inputs(x, w_qkv, w_proj, mask_np):
    """Build per-core input maps (host-side reshapes/permutes only)."""
    bf16 = ml_dtypes.bfloat16
    f8 = ml_dtypes.float8_e4m3
    xp = np.ascontiguousarray(x[:, PERM, :])                      # [B, N, C]
    xTp = np.ascontiguousarray(np.transpose(xp, (0, 2, 1)))       # [B, C, N]

    # bf16 x: per batch block [cc0 512 | cc1 512]
    xT_blk = xTp.reshape(B, 2, 128, N)                            # [B,cc,128,N]
    # fp8 x, DoubleRow pairs: block [i0 512 | i1 512], c = 2p+i
    x8_blk = xTp.reshape(B, 128, 2, N).transpose(0, 2, 1, 3)      # [B,i,128,N]

    # w8: [p, i*512 + f] = 16*w_qkv[f, 2p+i]  (DoubleRow channel pairs)
    W16 = (16.0 * np.asarray(w_qkv[:512])).astype(np.float32)     # [512, 256]
    w8 = np.ascontiguousarray(
        W16.T.reshape(128, 2, 512).reshape(128, 1024)
    ).astype(f8)

    wvT = np.ascontiguousarray(w_qkv[512:].T).reshape(2, 128, 256)
    wpT = np.ascontiguousarray(w_proj.T).reshape(2, 128, 256)
    wT = np.concatenate(
        [wvT[0], wpT[0], wvT[1], wpT[1]], axis=1
    ).astype(bf16)                                                # [128, 1024]

    consts = np.zeros((128, CW), dtype=np.float32)
    m01p = (mask_np[PERM][:, PERM] == 0.0)
    # fp8 mask-bias factors: bias = U.T@V (DoubleRow over 128 parts x 2)
    # term1 (plane 1, parts 0:8):  (1-mh)(hk, hq)
    # term2 (plane 0, parts 0:128): mh(hk, hq) * (1-mw)(wk, wq)
    U, V = -208.0, 208.0
    c8 = np.zeros((128, C8W), dtype=np.float32)
    u8 = c8[:, 0:256].reshape(128, 2, 128)
    for k in range(128):
        hk, s = k % 8, k // 8
        u8[(hk * 16 + s), 0, k] = U
        u8[hk, 1, k] = U
    for kc in range(NKC):
        v8 = c8[:, V8OFF[kc]:V8OFF[kc] + 2 * WP[kc]].reshape(128, 2, WP[kc])
        for qr in range(WP[kc]):
            q = QA[kc] + qr
            hq, wq = q % 8, q // 8
            for p in range(128):
                r, s2 = p // 16, p % 16
                wk = kc * 16 + s2
                if abs(r - hq) <= 3 and abs(wk - wq) > 5:
                    v8[p, 0, qr] = V
            for p in range(8):
                if abs(p - hq) > 3:
                    v8[p, 1, qr] = V
        # exact-factorization check against the reference mask
        bias = (u8[:, 0, :].T @ v8[:, 0, :] + u8[:, 1, :].T @ v8[:, 1, :])
        want = (U * V) * (~m01p[QA[kc]:QE[kc], 128 * kc:128 * kc + 128].T)
        assert np.array_equal(bias, want), f"mask factorization wrong kc={kc}"
    for qs in range(2):
        for half in range(2):
            for qcl in range(2):
                idx2 = (qs * 2 + half) * 2 + qcl
                for mm in range(128):
                    consts[qcl * 8 + half * 4 + mm // 32,
                           SELOFF + idx2 * 128 + mm] = 1.0
    consts[:, ONESCOFF] = 1.0
    consts[0, ONESROFF:ONESROFF + 512] = 1.0

    base = {
        "w8": w8,
        "wT": wT,
        "constsT": consts.astype(bf16),
        "c8": c8.astype(f8),
        "identT": np.eye(128, dtype=np.float32),
    }
    in_maps = []
    for core in range(NCORES):
        mp = dict(base)
        sl = slice(core * BPC, (core + 1) * BPC)
        xc = xT_blk[sl]                                           # [BPC,2,128,N]
        mp["xT"] = np.ascontiguousarray(
            xc.transpose(0, 2, 1, 3).reshape(BPC, 128, 1024)
            .transpose(1, 0, 2).reshape(128, BPC * 1024)
        ).astype(bf16)
        x8c = x8_blk[sl]                                          # [BPC,i,128,N]
        mp["x8"] = np.ascontiguousarray(
            x8c.transpose(2, 0, 1, 3).reshape(128, BPC * 1024)
        ).astype(f8)
        in_maps.append(mp)
    return in_maps


def run_sharded(x, w_qkv, w_proj, b_proj, mask, trace=False):
    """Compile+run on 8 cores; returns (out_full, BassKernelResults)."""
    from concourse.bass_utils import run_bass_kernel_spmd

    x = np.asarray(x, dtype=np.float32)
    w_qkv = np.asarray(w_qkv, dtype=np.float32)
    w_proj = np.asarray(w_proj, dtype=np.float32)
    b_proj = np.asarray(b_proj, dtype=np.float32)
    mask_np = np.asarray(mask, dtype=np.float32).reshape(N, N)

    nc = _build_nc()
    in_maps = _host_inputs(x, w_qkv, w_proj, mask_np)

    res = run_bass_kernel_spmd(nc, in_maps, core_ids=list(range(NCORES)), trace=trace)

    out_full = np.empty((B, N, C), dtype=np.float32)
    for core in range(NCORES):
        od = res.results[core]["out"]          # [BPC, N, C], permuted rows
        for bi in range(BPC):
            out_full[core * BPC + bi][PERM, :] = od[bi]
    out_full += b_proj[None, None, :]
    return out_full, res


def kernel(x, w_qkv, w_proj, b_proj, mask):
    out, _ = run_sharded(x, w_qkv, w_proj, b_proj, mask, trace=False)
    return out


# revision 5
# speedup vs baseline: 1.2177x; 1.0683x over previous
# Trainium2 Bass kernel for nn_AttentionBlock (local 7x11 windowed attention).
# V2: flattened cross-batch pipeline, PSUM bank parity, fp8-DoubleRow q/k path,
# tight exp APs, split DMA queues, norm/proj chain spread across tile slots.
import numpy as np
import ml_dtypes
from collections import deque

B, H, WG, C, HEADS = 32, 8, 64, 256, 8
HK, WK = 7, 11
N = H * WG              # 512
HD = C // HEADS         # 32
SCALE = float(HD) ** -0.5
NCORES = 8
BPC = B // NCORES       # 4
WT = 16                 # key-chunk width (grid cols)
NKC = WG // WT          # 4
HALO = WK // 2          # 5

# n' = w*8 + h  ->  n = h*64 + w
PERM = np.array([(i % H) * WG + (i // H) for i in range(N)], dtype=np.int64)


def _kc_qwin(kc):
    c0 = max(0, WT * kc - HALO)
    c1 = min(WG, WT * kc + WT + HALO)
    return c0 * H, c1 * H


QW = [_kc_qwin(kc) for kc in range(NKC)]          # real [qw0, qw1)
QA = [qw0 - (qw0 % 64) for qw0, _ in QW]          # aligned start
QE = [qw1 + (-qw1) % 64 for _, qw1 in QW]         # aligned end
WP = [QE[kc] - QA[kc] for kc in range(NKC)]       # padded width


def _segs(kc):
    return [(a, a + 64) for a in range(QA[kc], QE[kc], 64)]


SEGS = [_segs(kc) for kc in range(NKC)]
LASTKC = {}
for kc in range(NKC):
    for (sa, sb_) in SEGS[kc]:
        LASTKC[sa // 128] = kc

# packed-constant layout (columns of constsT [128, CW] bf16)
SELOFF = 0                # sel [16, 1024] at rows 0:16
ONESCOFF = SELOFF + 1024  # ones column [128, 1]
ONESROFF = ONESCOFF + 1   # ones row [1, 512] (row 0)
ZROWOFF = ONESROFF + 512  # zero row [1, 128] (row 0)
CW = ZROWOFF + 128

# fp8 mask-bias consts c8 [128, C8W]: U8 [128, 2, 128] then V8 per kc
# [128, 2, WP[kc]].  bias = U8.T (x) V8 accumulated into the scores PSUM via
# a zero-slot DoubleRow matmul; masked cells get -43264 raw (= -29.9 after
# the exp scale SCALE/256), unmasked cells get exactly 0.
V8OFF = []
_o8 = 256
for _kc in range(NKC):
    V8OFF.append(_o8)
    _o8 += 2 * WP[_kc]
C8W = _o8

_NC_CACHE = {}

# global tile indices (ti within batch) whose mask-multiply runs on GPSIMD
_POOL_MULS = {0, 2, 4, 6, 8, 10, 12, 14}

LAG = 5

_WAIT_CAPS = {
    k: 1
    for k in (
        "InstMatmult", "InstLdweights", "InstActivation", "InstTensorTensor",
        "InstTensorCopy", "InstDMACopy", "InstDrain", "InstCustomDveAnt",
        "InstTensorScalarPtr", "InstMemset", "InstTensorReduce",
        "InstReciprocal",
    )
}
_NOP_WAIT_CAP = 1


def _split_waits(nc):
    import concourse.mybir as mybir

    ctr = [0]
    for fn in nc.m.functions:
        for bb in fn.blocks:
            out = []
            for ins in bb.instructions:
                cap = _WAIT_CAPS.get(ins.__class__.__name__)
                si = getattr(ins, "sync_info", None)
                waits = list(si.on_wait) if si is not None else []
                if cap is not None and len(waits) > cap:
                    excess = waits[:-cap] if cap else waits
                    keep = waits[-cap:] if cap else []
                    while excess:
                        chunk = excess[:_NOP_WAIT_CAP]
                        excess = excess[_NOP_WAIT_CAP:]
                        w = mybir.InstEventSemaphore(
                            name=f"wsplit{ctr[0]}", ins=[], outs=[]
                        )
                        ctr[0] += 1
                        w.engine = ins.engine
                        w.sync_info = mybir.SyncInfo(
                            on_wait=chunk, on_update=[]
                        )
                        out.append(w)
                    ins.sync_info = mybir.SyncInfo(
                        on_wait=keep, on_update=list(si.on_update)
                    )
                out.append(ins)
            bb.instructions = out


def _build_nc(split_waits=True):
    key = ("nc", split_waits)
    if key in _NC_CACHE:
        return _NC_CACHE[key]
    import concourse.bass as bass
    import concourse.mybir as mybir
    import concourse.tile as tile
    import contextlib

    f32 = mybir.dt.float32
    bf16 = mybir.dt.bfloat16
    f8 = mybir.dt.float8e4
    EXP = mybir.ActivationFunctionType.Exp
    DR = mybir.MatmulPerfMode.DoubleRow

    nc = bass.Bass("TRN2")

    xTd = nc.dram_tensor("xT", [128, BPC * 1024], bf16, kind="ExternalInput")
    x8d = nc.dram_tensor("x8", [128, BPC * 1024], f8, kind="ExternalInput")
    w8d = nc.dram_tensor("w8", [128, 1024], f8, kind="ExternalInput")
    wTd = nc.dram_tensor("wT", [128, 1024], bf16, kind="ExternalInput")
    constsT = nc.dram_tensor("constsT", [128, CW], bf16, kind="ExternalInput")
    c8d = nc.dram_tensor("c8", [128, C8W], f8, kind="ExternalInput")
    identT = nc.dram_tensor("identT", [128, 128], f32, kind="ExternalInput")
    out = nc.dram_tensor("out", [BPC, N, C], f32, kind="ExternalOutput")

    with tile.TileContext(nc) as tc:
        with contextlib.ExitStack() as ctx:
            singles = ctx.enter_context(tc.tile_pool(name="singles", bufs=1))
            sb = ctx.enter_context(tc.tile_pool(name="sb", bufs=2))
            psp = ctx.enter_context(tc.tile_pool(name="ps", bufs=1, space="PSUM"))

            # ---- PSUM banks: 3 rotating + 4 avT (batch parity) + 1 shared ----
            rot = [psp.tile([128, 512], f32, name=f"rot{i}") for i in range(3)]
            avb = [psp.tile([128, 512], f32, name=f"avp{i}") for i in range(4)]
            sums_t = psp.tile([128, 288], f32, name="sums")
            rotc = [0]

            def nxt_rot():
                t = rot[rotc[0] % 3]
                rotc[0] += 1
                return t

            # ---- singles + DMAs (two queues: sync critical, scalar bg) ----
            s_w8 = singles.tile([128, 1024], f8, name="s_w8")
            s_wT = singles.tile([128, 1024], bf16, name="s_wT")
            s_x8 = singles.tile([128, BPC * 1024], f8, name="s_x8")
            s_xT = singles.tile([128, BPC * 1024], bf16, name="s_xT")
            s_consts = singles.tile([128, CW], bf16, name="s_consts")
            s_c8 = singles.tile([128, C8W], f8, name="s_c8")
            s_ident = singles.tile([128, 128], f32, name="s_ident")

            nc.sync.dma_start(out=s_w8, in_=w8d[:, :])
            nc.sync.dma_start(out=s_x8[:, 0:1024], in_=x8d[:, 0:1024])
            nc.sync.dma_start(
                out=s_consts[:, ONESCOFF:CW], in_=constsT[:, ONESCOFF:CW]
            )
            nc.sync.dma_start(out=s_c8, in_=c8d[:, :])
            nc.sync.dma_start(out=s_wT, in_=wTd[:, :])
            nc.sync.dma_start(out=s_xT[:, 0:1024], in_=xTd[:, 0:1024])
            nc.sync.dma_start(out=s_x8[:, 1024:], in_=x8d[:, 1024:])
            nc.sync.dma_start(out=s_xT[:, 1024:], in_=xTd[:, 1024:])
            nc.scalar.dma_start(
                out=s_consts[:, 0:ONESCOFF], in_=constsT[:, 0:ONESCOFF]
            )
            nc.scalar.dma_start(out=s_ident, in_=identT[:, :])

            s_u8 = s_c8[:, 0:256].rearrange("p (i k) -> p i k", i=2)
            s_v8 = {
                kc: s_c8[:, V8OFF[kc]: V8OFF[kc] + 2 * WP[kc]].rearrange(
                    "p (i q) -> p i q", i=2
                )
                for kc in range(NKC)
            }
            s_sel = s_consts[0:16, SELOFF:SELOFF + 1024]
            s_onesc = s_consts[:, ONESCOFF:ONESCOFF + 1]
            s_onesr = s_consts[0:1, ONESROFF:ONESROFF + 512]
            s_zrow = s_consts[0:1, ZROWOFF:ZROWOFF + 128]
            s_wv = [s_wT[:, cc * 512: cc * 512 + 256] for cc in range(2)]
            s_wp = [s_wT[:, cc * 512 + 256: cc * 512 + 512] for cc in range(2)]

            st = {}  # per-batch state: qk8, v, avT_sb

            def qkv_piece(b, piece):
                x8r = s_x8.rearrange("p (b i n) -> p b i n", b=BPC, i=2)
                w8r = s_w8.rearrange("p (i f) -> p i f", i=2)
                if piece == "q8":
                    st[b] = {"v": None, "avT_sb": None}
                    st[b]["qk8"] = sb.tile(
                        [128, 3072], f8, tag="qk8", bufs=2, name=f"qk8_{b}"
                    )
                    if b < 2:
                        # zero plane read by the DoubleRow scores matmuls
                        nc.gpsimd.memset(st[b]["qk8"][:, 2048:3072], 0.0)
                if piece in ("q8", "k8"):
                    base = 0 if piece == "q8" else 2
                    for P in (base, base + 1):
                        pt = nxt_rot()
                        nc.tensor.matmul(
                            pt[:, 0:512],
                            lhsT=w8r[:, :, P * 128:(P + 1) * 128],
                            rhs=x8r[:, b],
                            start=True, stop=True, perf_mode=DR,
                        )
                        nc.vector.tensor_copy(
                            st[b]["qk8"][:, P * 512:(P + 1) * 512],
                            pt[:, 0:512],
                        )
                    # w8r free index f = P*128 + m maps to w_qkv row f
                else:
                    if piece == "v01":
                        st[b]["v"] = sb.tile(
                            [128, 1024], bf16, tag="v", bufs=2, name=f"v{b}"
                        )
                        k0 = 0
                    else:
                        k0 = 2
                    pt = nxt_rot()
                    for kcb in (k0, k0 + 1):
                        for cc in range(2):
                            nc.tensor.matmul(
                                pt[:, (kcb % 2) * 256:(kcb % 2) * 256 + 256],
                                lhsT=s_xT[:, b * 1024 + cc * 512 + kcb * 128:
                                          b * 1024 + cc * 512 + kcb * 128 + 128],
                                rhs=s_wv[cc][:, :],
                                start=(cc == 0), stop=(cc == 1),
                            )
                    nc.vector.tensor_copy(
                        st[b]["v"][:, k0 * 256:k0 * 256 + 512], pt[:, 0:512]
                    )

            def sums_preclear():
                # covers the sums cols AND the two rT transpose slots
                nc.tensor.matmul(
                    sums_t[:, :], lhsT=s_zrow[:, :], rhs=s_onesr[:, 0:288],
                    start=True, stop=True, skip_group_check=True,
                )

            def preclears(b):
                par = (b % 2) * 2
                for i in range(2):
                    nc.tensor.matmul(
                        avb[par + i][:, :], lhsT=s_zrow[:, :], rhs=s_onesr[:, :],
                        start=True, stop=True, skip_group_check=True,
                    )
                st[b]["avT_sb"] = [
                    sb.tile([128, 512], bf16, tag="av", bufs=6, name=f"av{b}_{i}")
                    for i in range(2)
                ]

            tiles = [(kc, g) for kc in range(NKC) for g in range(4)]

            def tile_front(b, ti):
                kc, g = tiles[ti]
                qa0 = QA[kc]
                qw0, qw1 = QW[kc]
                pad = qw0 - qa0
                Wq = qw1 - qw0
                Wp = WP[kc]
                p_s = nxt_rot()
                r3 = st[b]["qk8"].rearrange("p (pl c) -> p pl c", pl=3)
                for i in range(2):
                    # additive mask bias via exact fp8 DoubleRow factorization
                    # (tight: the exp only reads [pad:pad+Wq])
                    nc.tensor.matmul(
                        p_s[:, i * 256 + pad: i * 256 + pad + Wq],
                        lhsT=s_u8,
                        rhs=s_v8[kc][:, :, pad:pad + Wq],
                        start=True, stop=False,
                        tile_position=(0, 0), perf_mode=DR,
                        skip_group_check=True,
                    )
                    # lhsT planes: (k block fc=2+i, zeros); rhs planes:
                    # (q block fc=i, next block x 0) -- zero-padded DoubleRow
                    nc.tensor.matmul(
                        p_s[:, i * 256 + pad: i * 256 + pad + Wq],
                        lhsT=r3[32 * g:32 * g + 32, 1:3,
                                i * 512 + kc * 128:i * 512 + kc * 128 + 128],
                        rhs=r3[32 * g:32 * g + 32, 0:2,
                               i * 512 + qw0:i * 512 + qw0 + Wq],
                        start=False, stop=True,
                        tile_position=(32 * g, 0), perf_mode=DR,
                        skip_group_check=True,
                    )
                e_t = sb.tile([128, 2 * Wp], bf16, tag="eT", bufs=8, name=f"eT{b}_{ti}")
                nc.scalar.activation(
                    e_t.rearrange("p (j s) -> p j s", j=2),
                    p_s.rearrange("p (j s) -> p j s", j=2)[:, :, :Wp],
                    EXP, scale=SCALE / 256.0,
                )
                return (b, kc, g, e_t)

            def consume(b, kc, g, p_t):
                par = (b % 2) * 2
                soff = (b % 2) * 32
                qw0, qw1 = QW[kc]
                qa0 = QA[kc]
                Wq = qw1 - qw0
                Wp = WP[kc]
                pad = qw0 - qa0
                for i in range(2):
                    h = g + 4 * i
                    j = g
                    nc.tensor.matmul(
                        avb[par + i][32 * j:32 * j + 32, qw0:qw1],
                        lhsT=st[b]["v"][:, kc * 256 + h * 32:
                                        kc * 256 + (h + 1) * 32],
                        rhs=p_t[:, i * Wp + pad:(i * Wp) + pad + Wq],
                        start=False, stop=(kc == NKC - 1),
                        tile_position=(0, 32 * j),
                        skip_group_check=True,
                    )
                    for (sa, sbnd) in SEGS[kc]:
                        qc = sa // 128
                        qcol = qc * 8 + h
                        nc.tensor.matmul(
                            sums_t[sa % 128: sa % 128 + 64,
                                   qcol:qcol + 1],
                            lhsT=p_t[:, i * Wp + (sa - qa0):
                                     i * Wp + (sbnd - qa0)],
                            rhs=s_onesc[:, :],
                            start=False, stop=(kc == LASTKC[qc]),
                            tile_position=(0, sa % 128),
                            skip_group_check=True,
                        )

            def norm_steps(b, qs):
                par = (b % 2) * 2
                rtc = 32 + ((2 * b + qs) % 2) * 128
                ns = {}

                def s_recip():
                    ns["r_q"] = sb.tile([128, 16], f32, tag="rq", bufs=6, name=f"rq{b}_{qs}")
                    nc.vector.reciprocal(
                        ns["r_q"], sums_t[:, qs * 16:qs * 16 + 16]
                    )

                def s_tp():
                    # into the sums bank's spare cols (precleared per batch);
                    # start=False: replace-onto-pending-zero
                    nc.tensor.matmul(
                        sums_t[0:16, rtc:rtc + 128], lhsT=ns["r_q"],
                        rhs=s_ident, is_transpose=True,
                        start=False, stop=True, skip_group_check=True,
                    )

                def s_rtcopy():
                    ns["rT_sb"] = sb.tile([16, 128], bf16, tag="rT", bufs=6, name=f"rT{b}_{qs}")
                    nc.vector.tensor_copy(ns["rT_sb"], sums_t[0:16, rtc:rtc + 128])

                def s_selmm():
                    ns["p_rb"] = nxt_rot()
                    for half in range(2):
                        for qcl in range(2):
                            idx = (qs * 2 + half) * 2 + qcl
                            nc.tensor.matmul(
                                ns["p_rb"][:, half * 256 + qcl * 128:
                                           half * 256 + qcl * 128 + 128],
                                lhsT=s_sel[:, idx * 128:idx * 128 + 128],
                                rhs=ns["rT_sb"],
                                start=True, stop=True,
                            )

                def s_rbcopy():
                    ns["rb_sb"] = sb.tile(
                        [128, 512], bf16, tag="rb", bufs=6, name=f"rb{b}_{qs}"
                    )
                    nc.vector.tensor_copy(
                        ns["rb_sb"][:, 0:256], ns["p_rb"][:, 0:256]
                    )
                    nc.scalar.copy(ns["rb_sb"][:, 256:512], ns["p_rb"][:, 256:512])

                def s_mul(half):
                    nc.vector.tensor_mul(
                        st[b]["avT_sb"][half][:, qs * 256:qs * 256 + 256],
                        ns["rb_sb"][:, half * 256:half * 256 + 256],
                        avb[par + half][:, qs * 256:qs * 256 + 256],
                    )

                def s_proj(qcl):
                    qc = 2 * qs + qcl
                    ns[f"p_o{qcl}"] = nxt_rot()
                    for half in range(2):
                        nc.tensor.matmul(
                            ns[f"p_o{qcl}"][:, 0:256],
                            lhsT=st[b]["avT_sb"][half][:, qc * 128:
                                                       (qc + 1) * 128],
                            rhs=s_wp[half][:, :],
                            start=(half == 0), stop=(half == 1),
                        )

                def s_out(qcl, on_act):
                    qc = 2 * qs + qcl
                    o_sb = sb.tile([128, 256], f32, tag="osb", bufs=6, name=f"osb{b}_{qs}_{qcl}")
                    if on_act:
                        nc.scalar.copy(o_sb, ns[f"p_o{qcl}"][:, 0:256])
                    else:
                        nc.vector.tensor_copy(o_sb, ns[f"p_o{qcl}"][:, 0:256])
                    nc.sync.dma_start(
                        out=out[b, qc * 128:(qc + 1) * 128, :], in_=o_sb
                    )

                return [
                    s_recip, s_tp, s_rtcopy, s_selmm, s_rbcopy,
                    lambda: s_mul(0), lambda: s_mul(1),
                    lambda: s_proj(0), lambda: s_out(0, qs == 1),
                    lambda: s_proj(1), lambda: s_out(1, False),
                ]

            # ---- prologue: batch 0 qkv + preclears ----
            qkv_piece(0, "q8")
            qkv_piece(0, "k8")
            preclears(0)
            sums_preclear()

            # ---- flat pipeline over all (batch, tile) slots ----
            pend = []
            normq = deque()
            T_total = BPC * 16
            for T in range(T_total + LAG):
                if T < T_total:
                    b, ti = divmod(T, 16)
                    pend.append(tile_front(b, ti))
                    if b == 0:
                        if ti == 1:
                            qkv_piece(0, "v01")
                        elif ti == 3:
                            qkv_piece(0, "v23")
                    nb = b + 1
                    if nb < BPC:
                        if ti == 5:
                            qkv_piece(nb, "q8")
                        elif ti == 7:
                            qkv_piece(nb, "k8")
                        elif ti == 9:
                            qkv_piece(nb, "v01")
                        elif ti == 11:
                            qkv_piece(nb, "v23")
                        elif ti == 13:
                            preclears(nb)
                # drain BEFORE consume: keeps stream order around the
                # sums preclear; 2/slot when backlogged
                k = 2 if len(normq) > 6 else 1
                for _ in range(min(k, len(normq))):
                    normq.popleft()()
                if T >= LAG:
                    consume(*pend[T - LAG])
                    cb, cti = divmod(T - LAG, 16)
                    if cti == 11:
                        normq.extend(norm_steps(cb, 0))
                    elif cti == 15:
                        # strict stream order: recip(qs1) reads the sums
                        # bank, then the preclear for batch cb+1 rewrites it,
                        # then (next slot) consume(cb+1, 0) accumulates
                        steps = norm_steps(cb, 1)
                        steps[0]()
                        # always: re-arms the rT transpose slots (start=False
                        # transposes rely on pending-zero from this preclear)
                        sums_preclear()
                        normq.extend(steps[1:])
            while normq:
                normq.popleft()()

    if split_waits:
        _split_waits(nc)
    _NC_CACHE[key] = nc
    return nc


def _host_inputs(x, w_qkv, w_proj, mask_np):
    """Build per-core input maps (host-side reshapes/permutes only)."""
    bf16 = ml_dtypes.bfloat16
    f8 = ml_dtypes.float8_e4m3
    xp = np.ascontiguousarray(x[:, PERM, :])                      # [B, N, C]
    xTp = np.ascontiguousarray(np.transpose(xp, (0, 2, 1)))       # [B, C, N]

    # bf16 x: per batch block [cc0 512 | cc1 512]
    xT_blk = xTp.reshape(B, 2, 128, N)                            # [B,cc,128,N]
    # fp8 x, DoubleRow pairs: block [i0 512 | i1 512], c = 2p+i
    x8_blk = xTp.reshape(B, 128, 2, N).transpose(0, 2, 1, 3)      # [B,i,128,N]

    # w8: [p, i*512 + f] = 16*w_qkv[f, 2p+i]  (DoubleRow channel pairs)
    W16 = (16.0 * np.asarray(w_qkv[:512])).astype(np.float32)     # [512, 256]
    w8 = np.ascontiguousarray(
        W16.T.reshape(128, 2, 512).reshape(128, 1024)
    ).astype(f8)

    wvT = np.ascontiguousarray(w_qkv[512:].T).reshape(2, 128, 256)
    wpT = np.ascontiguousarray(w_proj.T).reshape(2, 128, 256)
    wT = np.concatenate(
        [wvT[0], wpT[0], wvT[1], wpT[1]], axis=1
    ).astype(bf16)                                                # [128, 1024]

    consts = np.zeros((128, CW), dtype=np.float32)
    m01p = (mask_np[PERM][:, PERM] == 0.0)
    # fp8 mask-bias factors: bias = U.T@V (DoubleRow over 128 parts x 2)
    # term1 (plane 1, parts 0:8):  (1-mh)(hk, hq)
    # term2 (plane 0, parts 0:128): mh(hk, hq) * (1-mw)(wk, wq)
    U, V = -208.0, 208.0
    c8 = np.zeros((128, C8W), dtype=np.float32)
    u8 = c8[:, 0:256].reshape(128, 2, 128)
    for k in range(128):
        hk, s = k % 8, k // 8
        u8[(hk * 16 + s), 0, k] = U
        u8[hk, 1, k] = U
    for kc in range(NKC):
        v8 = c8[:, V8OFF[kc]:V8OFF[kc] + 2 * WP[kc]].reshape(128, 2, WP[kc])
        for qr in range(WP[kc]):
            q = QA[kc] + qr
            hq, wq = q % 8, q // 8
            for p in range(128):
                r, s2 = p // 16, p % 16
                wk = kc * 16 + s2
                if abs(r - hq) <= 3 and abs(wk - wq) > 5:
                    v8[p, 0, qr] = V
            for p in range(8):
                if abs(p - hq) > 3:
                    v8[p, 1, qr] = V
        # exact-factorization check against the reference mask
        bias = (u8[:, 0, :].T @ v8[:, 0, :] + u8[:, 1, :].T @ v8[:, 1, :])
        want = (U * V) * (~m01p[QA[kc]:QE[kc], 128 * kc:128 * kc + 128].T)
        assert np.array_equal(bias, want), f"mask factorization wrong kc={kc}"
    for qs in range(2):
        for half in range(2):
            for qcl in range(2):
                idx2 = (qs * 2 + half) * 2 + qcl
                for mm in range(128):
                    consts[qcl * 8 + half * 4 + mm // 32,
                           SELOFF + idx2 * 128 + mm] = 1.0
    consts[:, ONESCOFF] = 1.0
    consts[0, ONESROFF:ONESROFF + 512] = 1.0

    base = {
        "w8": w8,
        "wT": wT,
        "constsT": consts.astype(bf16),
        "c8": c8.astype(f8),
        "identT": np.eye(128, dtype=np.float32),
    }
    in_maps = []
    for core in range(NCORES):
        mp = dict(base)
        sl = slice(core * BPC, (core + 1) * BPC)
        xc = xT_blk[sl]                                           # [BPC,2,128,N]
        mp["xT"] = np.ascontiguousarray(
            xc.transpose(0, 2, 1, 3).reshape(BPC, 128, 1024)
            .transpose(1, 0, 2).reshape(128, BPC * 1024)
        ).astype(bf16)
        x8c = x8_blk[sl]                                          # [BPC,i,128,N]
        mp["x8"] = np.ascontiguousarray(
            x8c.transpose(2, 0, 1, 3).reshape(128, BPC * 1024)
        ).astype(f8)
        in_maps.append(mp)
    return in_maps


def run_sharded(x, w_qkv, w_proj, b_proj, mask, trace=False):
    """Compile+run on 8 cores; returns (out_full, BassKernelResults)."""
    from concourse.bass_utils import run_bass_kernel_spmd

    x = np.asarray(x, dtype=np.float32)
    w_qkv = np.asarray(w_qkv, dtype=np.float32)
    w_proj = np.asarray(w_proj, dtype=np.float32)
    b_proj = np.asarray(b_proj, dtype=np.float32)
    mask_np = np.asarray(mask, dtype=np.float32).reshape(N, N)

    nc = _build_nc()
    in_maps = _host_inputs(x, w_qkv, w_proj, mask_np)

    res = run_bass_kernel_spmd(nc, in_maps, core_ids=list(range(NCORES)), trace=trace)

    out_full = np.empty((B, N, C), dtype=np.float32)
    for core in range(NCORES):
        od = res.results[core]["out"]          # [BPC, N, C], permuted rows
        for bi in range(BPC):
            out_full[core * BPC + bi][PERM, :] = od[bi]
    out_full += b_proj[None, None, :]
    return out_full, res


def kernel(x, w_qkv, w_proj, b_proj, mask):
    out, _ = run_sharded(x, w_qkv, w_proj, b_proj, mask, trace=False)
    return out


# revision 6
# speedup vs baseline: 1.2272x; 1.0078x over previous
# Trainium2 Bass kernel for nn_AttentionBlock (local 7x11 windowed attention).
# V2: flattened cross-batch pipeline, PSUM bank parity, fp8-DoubleRow q/k path,
# tight exp APs, split DMA queues, norm/proj chain spread across tile slots.
import numpy as np
import ml_dtypes
from collections import deque

B, H, WG, C, HEADS = 32, 8, 64, 256, 8
HK, WK = 7, 11
N = H * WG              # 512
HD = C // HEADS         # 32
SCALE = float(HD) ** -0.5
NCORES = 8
BPC = B // NCORES       # 4
WT = 16                 # key-chunk width (grid cols)
NKC = WG // WT          # 4
HALO = WK // 2          # 5

# n' = w*8 + h  ->  n = h*64 + w
PERM = np.array([(i % H) * WG + (i // H) for i in range(N)], dtype=np.int64)


def _kc_qwin(kc):
    c0 = max(0, WT * kc - HALO)
    c1 = min(WG, WT * kc + WT + HALO)
    return c0 * H, c1 * H


QW = [_kc_qwin(kc) for kc in range(NKC)]          # real [qw0, qw1)
QA = [qw0 - (qw0 % 64) for qw0, _ in QW]          # aligned start
QE = [qw1 + (-qw1) % 64 for _, qw1 in QW]         # aligned end
WP = [QE[kc] - QA[kc] for kc in range(NKC)]       # padded width


def _segs(kc):
    return [(a, a + 64) for a in range(QA[kc], QE[kc], 64)]


SEGS = [_segs(kc) for kc in range(NKC)]
LASTKC = {}
for kc in range(NKC):
    for (sa, sb_) in SEGS[kc]:
        LASTKC[sa // 128] = kc

# packed-constant layout (columns of constsT [128, CW] bf16)
SELOFF = 0                # sel [16, 1024] at rows 0:16
ONESCOFF = SELOFF + 1024  # ones column [128, 1]
ONESROFF = ONESCOFF + 1   # ones row [1, 512] (row 0)
ZROWOFF = ONESROFF + 512  # zero row [1, 128] (row 0)
CW = ZROWOFF + 128

# fp8 mask-bias consts c8 [128, C8W]: U8 [128, 2, 128] then V8 per kc
# [128, 2, WP[kc]].  bias = U8.T (x) V8 accumulated into the scores PSUM via
# a zero-slot DoubleRow matmul; masked cells get -43264 raw (= -29.9 after
# the exp scale SCALE/256), unmasked cells get exactly 0.
V8OFF = []
_o8 = 256
for _kc in range(NKC):
    V8OFF.append(_o8)
    _o8 += 2 * WP[_kc]
C8W = _o8

_NC_CACHE = {}

# global tile indices (ti within batch) whose mask-multiply runs on GPSIMD
_POOL_MULS = {0, 2, 4, 6, 8, 10, 12, 14}

LAG = 5

_WAIT_CAPS = {
    k: 1
    for k in (
        "InstMatmult", "InstLdweights", "InstActivation", "InstTensorTensor",
        "InstTensorCopy", "InstDMACopy", "InstDrain", "InstCustomDveAnt",
        "InstTensorScalarPtr", "InstMemset", "InstTensorReduce",
        "InstReciprocal",
    )
}
_NOP_WAIT_CAP = 1


def _split_waits(nc):
    import concourse.mybir as mybir

    ctr = [0]
    for fn in nc.m.functions:
        for bb in fn.blocks:
            out = []
            for ins in bb.instructions:
                cap = _WAIT_CAPS.get(ins.__class__.__name__)
                si = getattr(ins, "sync_info", None)
                waits = list(si.on_wait) if si is not None else []
                if cap is not None and len(waits) > cap:
                    excess = waits[:-cap] if cap else waits
                    keep = waits[-cap:] if cap else []
                    while excess:
                        chunk = excess[:_NOP_WAIT_CAP]
                        excess = excess[_NOP_WAIT_CAP:]
                        w = mybir.InstEventSemaphore(
                            name=f"wsplit{ctr[0]}", ins=[], outs=[]
                        )
                        ctr[0] += 1
                        w.engine = ins.engine
                        w.sync_info = mybir.SyncInfo(
                            on_wait=chunk, on_update=[]
                        )
                        out.append(w)
                    ins.sync_info = mybir.SyncInfo(
                        on_wait=keep, on_update=list(si.on_update)
                    )
                out.append(ins)
            bb.instructions = out


def _build_nc(split_waits=True):
    key = ("nc", split_waits)
    if key in _NC_CACHE:
        return _NC_CACHE[key]
    import concourse.bass as bass
    import concourse.mybir as mybir
    import concourse.tile as tile
    import contextlib

    f32 = mybir.dt.float32
    bf16 = mybir.dt.bfloat16
    f8 = mybir.dt.float8e4
    EXP = mybir.ActivationFunctionType.Exp
    DR = mybir.MatmulPerfMode.DoubleRow

    nc = bass.Bass("TRN2")

    xTd = nc.dram_tensor("xT", [128, BPC * 1024], bf16, kind="ExternalInput")
    x8d = nc.dram_tensor("x8", [128, BPC * 1024], f8, kind="ExternalInput")
    w8d = nc.dram_tensor("w8", [128, 1024], f8, kind="ExternalInput")
    wTd = nc.dram_tensor("wT", [128, 1024], bf16, kind="ExternalInput")
    constsT = nc.dram_tensor("constsT", [128, CW], bf16, kind="ExternalInput")
    c8d = nc.dram_tensor("c8", [128, C8W], f8, kind="ExternalInput")
    identT = nc.dram_tensor("identT", [128, 128], f32, kind="ExternalInput")
    out = nc.dram_tensor("out", [BPC, N, C], f32, kind="ExternalOutput")

    with tile.TileContext(nc) as tc:
        with contextlib.ExitStack() as ctx:
            singles = ctx.enter_context(tc.tile_pool(name="singles", bufs=1))
            sb = ctx.enter_context(tc.tile_pool(name="sb", bufs=2))
            psp = ctx.enter_context(tc.tile_pool(name="ps", bufs=1, space="PSUM"))

            # ---- PSUM banks: 3 rotating + 4 avT (batch parity) + 1 shared ----
            rot = [psp.tile([128, 512], f32, name=f"rot{i}") for i in range(3)]
            avb = [psp.tile([128, 512], f32, name=f"avp{i}") for i in range(4)]
            sums_t = psp.tile([128, 288], f32, name="sums")
            rotc = [0]

            def nxt_rot():
                t = rot[rotc[0] % 3]
                rotc[0] += 1
                return t

            # ---- singles + DMAs (two queues: sync critical, scalar bg) ----
            s_w8 = singles.tile([128, 1024], f8, name="s_w8")
            s_wT = singles.tile([128, 1024], bf16, name="s_wT")
            s_x8 = singles.tile([128, BPC * 1024], f8, name="s_x8")
            s_xT = singles.tile([128, BPC * 1024], bf16, name="s_xT")
            s_consts = singles.tile([128, CW], bf16, name="s_consts")
            s_c8 = singles.tile([128, C8W], f8, name="s_c8")
            s_ident = singles.tile([128, 128], f32, name="s_ident")

            nc.sync.dma_start(out=s_w8, in_=w8d[:, :])
            nc.sync.dma_start(out=s_x8[:, 0:1024], in_=x8d[:, 0:1024])
            nc.sync.dma_start(
                out=s_consts[:, ONESCOFF:CW], in_=constsT[:, ONESCOFF:CW]
            )
            nc.sync.dma_start(out=s_c8, in_=c8d[:, :])
            nc.sync.dma_start(out=s_wT, in_=wTd[:, :])
            nc.sync.dma_start(out=s_xT[:, 0:1024], in_=xTd[:, 0:1024])
            nc.sync.dma_start(out=s_x8[:, 1024:], in_=x8d[:, 1024:])
            nc.sync.dma_start(out=s_xT[:, 1024:], in_=xTd[:, 1024:])
            nc.scalar.dma_start(
                out=s_consts[:, 0:ONESCOFF], in_=constsT[:, 0:ONESCOFF]
            )
            nc.scalar.dma_start(out=s_ident, in_=identT[:, :])

            s_u8 = s_c8[:, 0:256].rearrange("p (i k) -> p i k", i=2)
            s_v8 = {
                kc: s_c8[:, V8OFF[kc]: V8OFF[kc] + 2 * WP[kc]].rearrange(
                    "p (i q) -> p i q", i=2
                )
                for kc in range(NKC)
            }
            s_sel = s_consts[0:16, SELOFF:SELOFF + 1024]
            s_onesc = s_consts[:, ONESCOFF:ONESCOFF + 1]
            s_onesr = s_consts[0:1, ONESROFF:ONESROFF + 512]
            s_zrow = s_consts[0:1, ZROWOFF:ZROWOFF + 128]
            s_wv = [s_wT[:, cc * 512: cc * 512 + 256] for cc in range(2)]
            s_wp = [s_wT[:, cc * 512 + 256: cc * 512 + 512] for cc in range(2)]

            st = {}  # per-batch state: qk8, v, avT_sb

            def qkv_piece(b, piece):
                x8r = s_x8.rearrange("p (b i n) -> p b i n", b=BPC, i=2)
                w8r = s_w8.rearrange("p (i f) -> p i f", i=2)
                if piece == "q8":
                    st[b] = {"v": None, "avT_sb": None}
                    st[b]["qk8"] = sb.tile(
                        [128, 3072], f8, tag="qk8", bufs=2, name=f"qk8_{b}"
                    )
                    if b < 2:
                        # zero plane read by the DoubleRow scores matmuls
                        nc.gpsimd.memset(st[b]["qk8"][:, 2048:3072], 0.0)
                if piece in ("q8", "k8"):
                    base = 0 if piece == "q8" else 2
                    for P in (base, base + 1):
                        pt = nxt_rot()
                        nc.tensor.matmul(
                            pt[:, 0:512],
                            lhsT=w8r[:, :, P * 128:(P + 1) * 128],
                            rhs=x8r[:, b],
                            start=True, stop=True, perf_mode=DR,
                        )
                        nc.vector.tensor_copy(
                            st[b]["qk8"][:, P * 512:(P + 1) * 512],
                            pt[:, 0:512],
                        )
                    # w8r free index f = P*128 + m maps to w_qkv row f
                else:
                    if piece == "v01":
                        st[b]["v"] = sb.tile(
                            [128, 1024], bf16, tag="v", bufs=2, name=f"v{b}"
                        )
                        k0 = 0
                    else:
                        k0 = 2
                    pt = nxt_rot()
                    for kcb in (k0, k0 + 1):
                        for cc in range(2):
                            nc.tensor.matmul(
                                pt[:, (kcb % 2) * 256:(kcb % 2) * 256 + 256],
                                lhsT=s_xT[:, b * 1024 + cc * 512 + kcb * 128:
                                          b * 1024 + cc * 512 + kcb * 128 + 128],
                                rhs=s_wv[cc][:, :],
                                start=(cc == 0), stop=(cc == 1),
                            )
                    nc.vector.tensor_copy(
                        st[b]["v"][:, k0 * 256:k0 * 256 + 512], pt[:, 0:512]
                    )

            def sums_preclear():
                # covers the sums cols AND the two rT transpose slots
                nc.tensor.matmul(
                    sums_t[:, :], lhsT=s_zrow[:, :], rhs=s_onesr[:, 0:288],
                    start=True, stop=True, skip_group_check=True,
                )

            def preclears(b):
                par = (b % 2) * 2
                for i in range(2):
                    nc.tensor.matmul(
                        avb[par + i][:, :], lhsT=s_zrow[:, :], rhs=s_onesr[:, :],
                        start=True, stop=True, skip_group_check=True,
                    )
                st[b]["avT_sb"] = [
                    sb.tile([128, 512], bf16, tag="av", bufs=6, name=f"av{b}_{i}")
                    for i in range(2)
                ]

            tiles = [(kc, g) for kc in range(NKC) for g in range(4)]

            def tile_front(b, ti):
                kc, g = tiles[ti]
                qa0 = QA[kc]
                qw0, qw1 = QW[kc]
                pad = qw0 - qa0
                Wq = qw1 - qw0
                Wp = WP[kc]
                p_s = nxt_rot()
                r3 = st[b]["qk8"].rearrange("p (pl c) -> p pl c", pl=3)
                for i in range(2):
                    # additive mask bias via exact fp8 DoubleRow factorization
                    # (tight: the exp only reads [pad:pad+Wq])
                    nc.tensor.matmul(
                        p_s[:, i * 256 + pad: i * 256 + pad + Wq],
                        lhsT=s_u8,
                        rhs=s_v8[kc][:, :, pad:pad + Wq],
                        start=True, stop=False,
                        tile_position=(0, 0), perf_mode=DR,
                        skip_group_check=True,
                    )
                    # lhsT planes: (k block fc=2+i, zeros); rhs planes:
                    # (q block fc=i, next block x 0) -- zero-padded DoubleRow
                    nc.tensor.matmul(
                        p_s[:, i * 256 + pad: i * 256 + pad + Wq],
                        lhsT=r3[32 * g:32 * g + 32, 1:3,
                                i * 512 + kc * 128:i * 512 + kc * 128 + 128],
                        rhs=r3[32 * g:32 * g + 32, 0:2,
                               i * 512 + qw0:i * 512 + qw0 + Wq],
                        start=False, stop=True,
                        tile_position=(32 * g, 0), perf_mode=DR,
                        skip_group_check=True,
                    )
                e_t = sb.tile([128, 2 * Wp], bf16, tag="eT", bufs=8, name=f"eT{b}_{ti}")
                nc.scalar.activation(
                    e_t.rearrange("p (j s) -> p j s", j=2),
                    p_s.rearrange("p (j s) -> p j s", j=2)[:, :, :Wp],
                    EXP, scale=SCALE / 256.0,
                )
                return (b, kc, g, e_t)

            def consume(b, kc, g, p_t):
                par = (b % 2) * 2
                soff = (b % 2) * 32
                qw0, qw1 = QW[kc]
                qa0 = QA[kc]
                Wq = qw1 - qw0
                Wp = WP[kc]
                pad = qw0 - qa0
                for i in range(2):
                    h = g + 4 * i
                    j = g
                    nc.tensor.matmul(
                        avb[par + i][32 * j:32 * j + 32, qw0:qw1],
                        lhsT=st[b]["v"][:, kc * 256 + h * 32:
                                        kc * 256 + (h + 1) * 32],
                        rhs=p_t[:, i * Wp + pad:(i * Wp) + pad + Wq],
                        start=False, stop=(kc == NKC - 1),
                        tile_position=(0, 32 * j),
                        skip_group_check=True,
                    )
                    for (sa, sbnd) in SEGS[kc]:
                        qc = sa // 128
                        qcol = qc * 8 + h
                        nc.tensor.matmul(
                            sums_t[sa % 128: sa % 128 + 64,
                                   qcol:qcol + 1],
                            lhsT=p_t[:, i * Wp + (sa - qa0):
                                     i * Wp + (sbnd - qa0)],
                            rhs=s_onesc[:, :],
                            start=False, stop=(kc == LASTKC[qc]),
                            tile_position=(0, sa % 128),
                            skip_group_check=True,
                        )

            def norm_steps(b, qs):
                par = (b % 2) * 2
                rtc = 32 + ((2 * b + qs) % 2) * 128
                ns = {}

                def s_recip():
                    ns["r_q"] = sb.tile([128, 16], f32, tag="rq", bufs=6, name=f"rq{b}_{qs}")
                    nc.vector.reciprocal(
                        ns["r_q"], sums_t[:, qs * 16:qs * 16 + 16]
                    )

                def s_tp():
                    # into the sums bank's spare cols (precleared per batch);
                    # start=False: replace-onto-pending-zero
                    nc.tensor.matmul(
                        sums_t[0:16, rtc:rtc + 128], lhsT=ns["r_q"],
                        rhs=s_ident, is_transpose=True,
                        start=False, stop=True, skip_group_check=True,
                    )

                def s_rtcopy():
                    ns["rT_sb"] = sb.tile([16, 128], bf16, tag="rT", bufs=6, name=f"rT{b}_{qs}")
                    nc.vector.tensor_copy(ns["rT_sb"], sums_t[0:16, rtc:rtc + 128])

                def s_selmm():
                    ns["p_rb"] = nxt_rot()
                    for half in range(2):
                        for qcl in range(2):
                            idx = (qs * 2 + half) * 2 + qcl
                            nc.tensor.matmul(
                                ns["p_rb"][:, half * 256 + qcl * 128:
                                           half * 256 + qcl * 128 + 128],
                                lhsT=s_sel[:, idx * 128:idx * 128 + 128],
                                rhs=ns["rT_sb"],
                                start=True, stop=True,
                            )

                def s_rbcopy():
                    ns["rb_sb"] = sb.tile(
                        [128, 512], bf16, tag="rb", bufs=6, name=f"rb{b}_{qs}"
                    )
                    nc.vector.tensor_copy(
                        ns["rb_sb"][:, 0:256], ns["p_rb"][:, 0:256]
                    )
                    nc.scalar.copy(ns["rb_sb"][:, 256:512], ns["p_rb"][:, 256:512])

                def s_mul(half):
                    nc.vector.tensor_mul(
                        st[b]["avT_sb"][half][:, qs * 256:qs * 256 + 256],
                        ns["rb_sb"][:, half * 256:half * 256 + 256],
                        avb[par + half][:, qs * 256:qs * 256 + 256],
                    )

                def s_proj(qcl):
                    qc = 2 * qs + qcl
                    ns[f"p_o{qcl}"] = nxt_rot()
                    for half in range(2):
                        nc.tensor.matmul(
                            ns[f"p_o{qcl}"][:, 0:256],
                            lhsT=st[b]["avT_sb"][half][:, qc * 128:
                                                       (qc + 1) * 128],
                            rhs=s_wp[half][:, :],
                            start=(half == 0), stop=(half == 1),
                        )

                def s_out(qcl, on_act):
                    if qcl == 0:
                        ns["o_sb"] = sb.tile(
                            [128, 512], f32, tag="osb", bufs=6,
                            name=f"osb{b}_{qs}"
                        )
                    dst = ns["o_sb"][:, qcl * 256:qcl * 256 + 256]
                    if on_act:
                        nc.scalar.copy(dst, ns[f"p_o{qcl}"][:, 0:256])
                    else:
                        nc.vector.tensor_copy(dst, ns[f"p_o{qcl}"][:, 0:256])
                    if qcl == 1:
                        # one merged DMA for both 128-row output chunks
                        nc.sync.dma_start(
                            out=out[b, qs * 256:(qs + 1) * 256, :].rearrange(
                                "(q p) c -> p q c", q=2
                            ),
                            in_=ns["o_sb"].rearrange("p (q c) -> p q c", q=2),
                        )

                return [
                    s_recip, s_tp, s_rtcopy, s_selmm, s_rbcopy,
                    lambda: s_mul(0), lambda: s_mul(1),
                    lambda: s_proj(0), lambda: s_out(0, qs == 1),
                    lambda: s_proj(1), lambda: s_out(1, False),
                ]

            # ---- prologue: batch 0 qkv + preclears ----
            qkv_piece(0, "q8")
            qkv_piece(0, "k8")
            preclears(0)
            sums_preclear()

            # ---- flat pipeline over all (batch, tile) slots ----
            pend = []
            normq = deque()
            T_total = BPC * 16
            for T in range(T_total + LAG):
                if T < T_total:
                    b, ti = divmod(T, 16)
                    pend.append(tile_front(b, ti))
                    if b == 0:
                        if ti == 1:
                            qkv_piece(0, "v01")
                        elif ti == 3:
                            qkv_piece(0, "v23")
                    nb = b + 1
                    if nb < BPC:
                        if ti == 5:
                            qkv_piece(nb, "q8")
                        elif ti == 7:
                            qkv_piece(nb, "k8")
                        elif ti == 9:
                            qkv_piece(nb, "v01")
                        elif ti == 11:
                            qkv_piece(nb, "v23")
                        elif ti == 13:
                            preclears(nb)
                # drain BEFORE consume: keeps stream order around the
                # sums preclear; 2/slot when backlogged
                k = 2 if len(normq) > 6 else 1
                for _ in range(min(k, len(normq))):
                    normq.popleft()()
                if T >= LAG:
                    consume(*pend[T - LAG])
                    cb, cti = divmod(T - LAG, 16)
                    if cti == 11:
                        normq.extend(norm_steps(cb, 0))
                    elif cti == 15:
                        # strict stream order: recip(qs1) reads the sums
                        # bank, then the preclear for batch cb+1 rewrites it,
                        # then (next slot) consume(cb+1, 0) accumulates
                        steps = norm_steps(cb, 1)
                        steps[0]()
                        # always: re-arms the rT transpose slots (start=False
                        # transposes rely on pending-zero from this preclear)
                        sums_preclear()
                        normq.extend(steps[1:])
            while normq:
                normq.popleft()()

    if split_waits:
        _split_waits(nc)
    _NC_CACHE[key] = nc
    return nc


def _host_inputs(x, w_qkv, w_proj, mask_np):
    """Build per-core input maps (host-side reshapes/permutes only)."""
    bf16 = ml_dtypes.bfloat16
    f8 = ml_dtypes.float8_e4m3
    xp = np.ascontiguousarray(x[:, PERM, :])                      # [B, N, C]
    xTp = np.ascontiguousarray(np.transpose(xp, (0, 2, 1)))       # [B, C, N]

    # bf16 x: per batch block [cc0 512 | cc1 512]
    xT_blk = xTp.reshape(B, 2, 128, N)                            # [B,cc,128,N]
    # fp8 x, DoubleRow pairs: block [i0 512 | i1 512], c = 2p+i
    x8_blk = xTp.reshape(B, 128, 2, N).transpose(0, 2, 1, 3)      # [B,i,128,N]

    # w8: [p, i*512 + f] = 16*w_qkv[f, 2p+i]  (DoubleRow channel pairs)
    W16 = (16.0 * np.asarray(w_qkv[:512])).astype(np.float32)     # [512, 256]
    w8 = np.ascontiguousarray(
        W16.T.reshape(128, 2, 512).reshape(128, 1024)
    ).astype(f8)

    wvT = np.ascontiguousarray(w_qkv[512:].T).reshape(2, 128, 256)
    wpT = np.ascontiguousarray(w_proj.T).reshape(2, 128, 256)
    wT = np.concatenate(
        [wvT[0], wpT[0], wvT[1], wpT[1]], axis=1
    ).astype(bf16)                                                # [128, 1024]

    consts = np.zeros((128, CW), dtype=np.float32)
    m01p = (mask_np[PERM][:, PERM] == 0.0)
    # fp8 mask-bias factors: bias = U.T@V (DoubleRow over 128 parts x 2)
    # term1 (plane 1, parts 0:8):  (1-mh)(hk, hq)
    # term2 (plane 0, parts 0:128): mh(hk, hq) * (1-mw)(wk, wq)
    U, V = -208.0, 208.0
    c8 = np.zeros((128, C8W), dtype=np.float32)
    u8 = c8[:, 0:256].reshape(128, 2, 128)
    for k in range(128):
        hk, s = k % 8, k // 8
        u8[(hk * 16 + s), 0, k] = U
        u8[hk, 1, k] = U
    for kc in range(NKC):
        v8 = c8[:, V8OFF[kc]:V8OFF[kc] + 2 * WP[kc]].reshape(128, 2, WP[kc])
        for qr in range(WP[kc]):
            q = QA[kc] + qr
            hq, wq = q % 8, q // 8
            for p in range(128):
                r, s2 = p // 16, p % 16
                wk = kc * 16 + s2
                if abs(r - hq) <= 3 and abs(wk - wq) > 5:
                    v8[p, 0, qr] = V
            for p in range(8):
                if abs(p - hq) > 3:
                    v8[p, 1, qr] = V
        # exact-factorization check against the reference mask
        bias = (u8[:, 0, :].T @ v8[:, 0, :] + u8[:, 1, :].T @ v8[:, 1, :])
        want = (U * V) * (~m01p[QA[kc]:QE[kc], 128 * kc:128 * kc + 128].T)
        assert np.array_equal(bias, want), f"mask factorization wrong kc={kc}"
    for qs in range(2):
        for half in range(2):
            for qcl in range(2):
                idx2 = (qs * 2 + half) * 2 + qcl
                for mm in range(128):
                    consts[qcl * 8 + half * 4 + mm // 32,
                           SELOFF + idx2 * 128 + mm] = 1.0
    consts[:, ONESCOFF] = 1.0
    consts[0, ONESROFF:ONESROFF + 512] = 1.0

    base = {
        "w8": w8,
        "wT": wT,
        "constsT": consts.astype(bf16),
        "c8": c8.astype(f8),
        "identT": np.eye(128, dtype=np.float32),
    }
    in_maps = []
    for core in range(NCORES):
        mp = dict(base)
        sl = slice(core * BPC, (core + 1) * BPC)
        xc = xT_blk[sl]                                           # [BPC,2,128,N]
        mp["xT"] = np.ascontiguousarray(
            xc.transpose(0, 2, 1, 3).reshape(BPC, 128, 1024)
            .transpose(1, 0, 2).reshape(128, BPC * 1024)
        ).astype(bf16)
        x8c = x8_blk[sl]                                          # [BPC,i,128,N]
        mp["x8"] = np.ascontiguousarray(
            x8c.transpose(2, 0, 1, 3).reshape(128, BPC * 1024)
        ).astype(f8)
        in_maps.append(mp)
    return in_maps


def run_sharded(x, w_qkv, w_proj, b_proj, mask, trace=False):
    """Compile+run on 8 cores; returns (out_full, BassKernelResults)."""
    from concourse.bass_utils import run_bass_kernel_spmd

    x = np.asarray(x, dtype=np.float32)
    w_qkv = np.asarray(w_qkv, dtype=np.float32)
    w_proj = np.asarray(w_proj, dtype=np.float32)
    b_proj = np.asarray(b_proj, dtype=np.float32)
    mask_np = np.asarray(mask, dtype=np.float32).reshape(N, N)

    nc = _build_nc()
    in_maps = _host_inputs(x, w_qkv, w_proj, mask_np)

    res = run_bass_kernel_spmd(nc, in_maps, core_ids=list(range(NCORES)), trace=trace)

    out_full = np.empty((B, N, C), dtype=np.float32)
    for core in range(NCORES):
        od = res.results[core]["out"]          # [BPC, N, C], permuted rows
        for bi in range(BPC):
            out_full[core * BPC + bi][PERM, :] = od[bi]
    out_full += b_proj[None, None, :]
    return out_full, res


def kernel(x, w_qkv, w_proj, b_proj, mask):
    out, _ = run_sharded(x, w_qkv, w_proj, b_proj, mask, trace=False)
    return out
